# revision 1
# baseline (speedup 1.0000x reference)
"""RBF-kernel causal attention on 8 Trainium2 NeuronCores.

B=2, H=16, N=2048, D=64. Shards the 32 (b,h) attention instances across 8
cores (4 heads per core). Math notes:

  logits = -relu(||q-k||^2)/sqrt(D); relu is a no-op (||q-k||^2 >= 0 up to
  rounding), and softmax is invariant to per-query offsets, so
      softmax_n(-(qsq_m + ksq_n - 2 qk)/8) == softmax_n(qk/4 - ksq_n/8)
  We compute P'' = exp(0.25 * K Q^T) in a [key, query] layout and fold the
  exp(-0.125 ksq_n) per-key factor into V (and into the appended ones-column
  that produces the softmax denominator):
      [O^T | l] accumulates via matmul(lhsT=V_aug_scaled, rhs=P'').
  Final output O[m,d] = OT[d,m] / l[m], un-transposed via PE transpose.

Emission is manually software-pipelined: head h+1's setup chunks (transposes,
ksq, V scaling) are interleaved between head h's query blocks so the tile
scheduler (limited lookahead) can overlap them.
"""

import sys

if "/opt/trn_rl_repo" not in sys.path:
    sys.path.insert(0, "/opt/trn_rl_repo")

import numpy as np

import concourse.bacc as bacc
import concourse.mybir as mybir
import concourse.tile as tile
from concourse.masks import make_identity

B, H, N, D = 2, 16, 2048, 64
NCORES = 8
HPC = (B * H) // NCORES  # heads per core = 4
P = 128                  # partitions
NT = N // P              # key tiles per head = 16
QB = 512                 # query block (matmul moving dim)
MBS = N // QB            # query blocks per head = 4
G = 2                    # key tiles per exp/ACT group (2 PSUM banks)

F32 = mybir.dt.float32
# float32r = relaxed-precision fp32 matmul (1 cycle/row at moving dim >= 256
# instead of 4 for float32)
MM_DT = mybir.dt.float32r


def build_nc():
    nc = bacc.Bacc("TRN2", target_bir_lowering=False, debug=False)
    q = nc.dram_tensor("q", [HPC, N, D], F32, kind="ExternalInput")
    k = nc.dram_tensor("k", [HPC, N, D], F32, kind="ExternalInput")
    v = nc.dram_tensor("v", [HPC, N, D], F32, kind="ExternalInput")
    out = nc.dram_tensor("out", [HPC, N, D], F32, kind="ExternalOutput")

    with tile.TileContext(nc) as tc:
        with (
            tc.tile_pool(name="const", bufs=1) as const_pool,
            tc.tile_pool(name="loads", bufs=1) as load_pool,
            tc.tile_pool(name="head", bufs=2) as head_pool,
            tc.tile_pool(name="work", bufs=3) as work_pool,
            tc.tile_pool(name="p", bufs=4) as p_pool,
            tc.tile_pool(name="epi", bufs=3) as epi_pool,
            tc.tile_pool(name="st", bufs=3, space="PSUM") as st_pool,
            tc.tile_pool(name="otp", bufs=2, space="PSUM") as ot_pool,
        ):
            identity = const_pool.tile([P, P], F32)
            make_identity(nc, identity)
            # multiplicative causal masks for odd diagonal tiles (jj=1,3):
            # mask[jj][n, m] = 1.0 if m - n - 128*jj >= 0 else 0.0
            dmask = {}
            for jj in (1, 3):
                mk = const_pool.tile([P, QB], F32, tag=f"dmask{jj}", name="mk")
                nc.gpsimd.memset(mk[:], 1.0)
                nc.gpsimd.affine_select(
                    out=mk[:], in_=mk[:],
                    compare_op=mybir.AluOpType.is_ge, fill=0.0,
                    base=-P * jj, pattern=[[1, QB]], channel_multiplier=-1,
                )
                dmask[jj] = mk

            # prefetch every head's inputs up front: no-wait DMAs stream in
            # the background while compute proceeds
            knats, qnats, vtmps = [], [], []
            for h in range(HPC):
                # quarter-granular loads so the first transposes start as
                # soon as the first chunk lands, not after the whole head
                knat = load_pool.tile([P, NT, D], F32, tag=f"knat{h}")
                # q loaded DOUBLED along a repeat dim (two passes over DRAM):
                # transposing [128m, (2,64d)] then yields Q^T duplicated on
                # both partition halves, as the row-packed matmuls need
                qnat = load_pool.tile([P, NT, 2, D], F32, tag=f"qnat{h}")
                vtmp = load_pool.tile([P, NT, D], F32, tag=f"vtmp{h}")
                kq = k[h].rearrange("(t p) d -> p t d", p=P)
                qq = q[h].rearrange("(t p) d -> p t d", p=P)
                vq = v[h].rearrange("(t p) d -> p t d", p=P)
                nch = 4 if h == 0 else 1
                w_ = NT // nch
                for c in range(nch):
                    ts = slice(w_ * c, w_ * c + w_)
                    nc.sync.dma_start(knat[:, ts, :], kq[:, ts, :])
                    for r in range(2):
                        nc.sync.dma_start(qnat[:, ts, r, :], qq[:, ts, :])
                    nc.sync.dma_start(vtmp[:, ts, :], vq[:, ts, :])
                knats.append(knat)
                qnats.append(qnat)
                vtmps.append(vtmp)

            heads = [{} for _ in range(HPC)]

            def setup_chunks(h):
                """Emission chunks for head h's setup, in dependency order."""
                st = heads[h]

                def allocs():
                    st["ksq"] = head_pool.tile([P, NT], F32, tag="ksq", name="ksq")
                    st["w"] = head_pool.tile([P, NT], F32, tag="w", name="w")
                    st["vaug"] = head_pool.tile(
                        [P, NT, D + 1], MM_DT, tag="vaug", name="vaug"
                    )
                    # kt: key-tile PAIRS stacked on partition halves
                    # (even tile at partitions 0:64, odd at 64:128) so two
                    # QK matmuls can row-pack the PE array concurrently.
                    st["kt"] = head_pool.tile(
                        [P, NT // 2, P], MM_DT, tag="kt", name="kt"
                    )
                    # qt: Q^T duplicated into both partition halves (the
                    # row-packed matmuls stream rhs partitions 0:64 and
                    # 64:128 into array row groups 0-1 and 2-3)
                    st["qt"] = head_pool.tile([P, NT, P], MM_DT, tag="qt", name="qt")

                def scale_chunk(c, nt4=4):
                    # per-quarter V_aug build: runs as soon as that quarter
                    # of k and v has landed (head 0 only; later heads build
                    # whole-head to save per-instruction overhead)
                    def run():
                        ts = slice(4 * c, 4 * c + nt4)
                        knat, vtmp = knats[h], vtmps[h]
                        ksq, w, vaug = st["ksq"], st["w"], st["vaug"]
                        ktmp = work_pool.tile([P, nt4, D], F32, tag="ktmp")
                        nc.vector.tensor_mul(
                            out=ktmp[:], in0=knat[:, ts, :], in1=knat[:, ts, :]
                        )
                        nc.vector.tensor_reduce(
                            ksq[:, ts], ktmp[:],
                            axis=mybir.AxisListType.X, op=mybir.AluOpType.add,
                        )
                        nc.scalar.activation(
                            w[:, ts], ksq[:, ts],
                            mybir.ActivationFunctionType.Exp, scale=-0.125,
                        )
                        nc.gpsimd.tensor_mul(
                            out=vaug[:, ts, :D],
                            in0=vtmp[:, ts, :],
                            in1=w[:, ts, None].to_broadcast((P, nt4, D)),
                        )
                        nc.gpsimd.tensor_copy(
                            out=vaug[:, ts, D : D + 1], in_=w[:, ts, None]
                        )

                    return run

                def ktr_group(g):
                    # 4 pair-transposes: [128n, (2t, 64d)] -> [(2t, 64d), 128n]
                    # lands even tile at partitions 0:64, odd at 64:128
                    def run():
                        src = knats[h]
                        dst = heads[h]["kt"]
                        tp = st_pool.tile([P, 4, P], F32, tag="stg", name="tp")
                        for j in range(4):
                            pr = 4 * g + j
                            nc.tensor.transpose(
                                tp[:, j, :], src[:, 2 * pr : 2 * pr + 2, :],
                                identity[:],
                            )
                        nc.vector.tensor_copy(
                            out=dst[:, 4 * g : 4 * g + 4, :], in_=tp[:]
                        )

                    return run

                def qtr_group(g):
                    # transpose a 0-stride doubled view [128m, (2, 64d)] so
                    # the output holds Q^T duplicated on both partition
                    # halves (rows 0:64 and 64:128) in one shot
                    def run():
                        src = qnats[h]
                        dst = heads[h]["qt"]
                        tp = st_pool.tile([P, 4, P], F32, tag="stg", name="tp")
                        for j in range(4):
                            nc.tensor.transpose(
                                tp[:, j, :], src[:, 4 * g + j, :, :], identity[:]
                            )
                        nc.vector.tensor_copy(
                            out=dst[:, 4 * g : 4 * g + 4, :], in_=tp[:]
                        )

                    return run

                # query block mb needs kt pair-groups up to (2mb+1)//4, qt
                # group mb, and vaug quarter mb; yield in dependency order
                yield allocs
                if h == 0:
                    yield scale_chunk(0)
                    yield ktr_group(0)
                    yield qtr_group(0)
                    yield scale_chunk(1)
                    yield qtr_group(1)
                    yield ktr_group(1)
                    yield scale_chunk(2)
                    yield qtr_group(2)
                    yield scale_chunk(3)
                    yield qtr_group(3)
                else:
                    yield scale_chunk(0, NT)
                    yield ktr_group(0)
                    yield qtr_group(0)
                    yield qtr_group(1)
                    yield ktr_group(1)
                    yield qtr_group(2)
                    yield qtr_group(3)

            def job_chunks(h, mb):
                """Chunks of one (head, query-block) job, for interleaving."""
                kt, qt, vaug = heads[h]["kt"], heads[h]["qt"], heads[h]["vaug"]
                nsub = 4 * mb          # sub-diagonal key tiles
                qt_lo = qt[:D, 4 * mb : 4 * mb + 4, :]   # [64, 512]
                qt_hi = qt[D:, 4 * mb : 4 * mb + 4, :]   # [64, 512]
                ntiles = nsub + 4
                jst = {"prev": None, "ot": None}

                def sub_group(s):
                    def run():
                        if jst["ot"] is None:
                            jst["ot"] = ot_pool.tile(
                                [D + 1, QB], F32, tag="ot", name="ot"
                            )
                        stg = st_pool.tile([P, G, QB], F32, tag="stg")
                        pr = s // 2
                        nc.tensor.matmul(
                            stg[:, 0, :], kt[:D, pr, :], qt_lo,
                            start=True, stop=True, skip_group_check=True,
                        )
                        nc.tensor.matmul(
                            stg[:, 1, :], kt[D:, pr, :], qt_hi,
                            start=True, stop=True, skip_group_check=True,
                        )
                        pg = p_pool.tile([P, G, QB], MM_DT, tag="pg")
                        nc.scalar.activation(
                            pg[:], stg[:],
                            mybir.ActivationFunctionType.Exp, scale=0.25,
                        )
                        if jst["prev"] is not None:
                            _emit_pv(nc, jst["ot"], vaug, jst["prev"], ntiles)
                        jst["prev"] = (pg, [s, s + 1])

                    return run

                def diag_group(a):
                    def run():
                        if jst["ot"] is None:
                            jst["ot"] = ot_pool.tile(
                                [D + 1, QB], F32, tag="ot", name="ot"
                            )
                        if a == 0:
                            jst["pgd"] = p_pool.tile([P, 4, QB], MM_DT, tag="pgd", name="pgd")
                        pgd = jst["pgd"]
                        # columns m < 128*(2a) of tiles (2a, 2a+1) are fully
                        # masked: skip their QK matmul + exp; affine_select
                        # below zero-fills that (otherwise garbage) region.
                        c0 = P * 2 * a
                        stg = st_pool.tile([P, G, QB], F32, tag="stg")
                        pr = 2 * mb + a
                        nc.tensor.matmul(
                            stg[:, 0, c0:],
                            kt[:D, pr, :],
                            qt[:D, 4 * mb + 2 * a : 4 * mb + 4, :],
                            start=True, stop=True, skip_group_check=True,
                        )
                        nc.tensor.matmul(
                            stg[:, 1, c0:],
                            kt[D:, pr, :],
                            qt[D:, 4 * mb + 2 * a : 4 * mb + 4, :],
                            start=True, stop=True, skip_group_check=True,
                        )
                        nc.scalar.activation(
                            pgd[:, 2 * a : 2 * a + 2, c0:], stg[:, :, c0:],
                            mybir.ActivationFunctionType.Exp, scale=0.25,
                        )
                        # keep pgd[n, jj, m] iff m - n - 128 jj >= 0; the
                        # even tile masks on Pool (affine_select), the odd
                        # ones concurrently on DVE (mul by a const mask)
                        jj = 2 * a
                        nc.gpsimd.affine_select(
                            out=pgd[:, jj, :], in_=pgd[:, jj, :],
                            compare_op=mybir.AluOpType.is_ge, fill=0.0,
                            base=-P * jj, pattern=[[1, QB]],
                            channel_multiplier=-1,
                        )
                        nc.vector.tensor_mul(
                            out=pgd[:, jj + 1, :],
                            in0=pgd[:, jj + 1, :],
                            in1=dmask[jj + 1][:],
                        )

                    return run

                def pv_epilogue():
                    ot, pgd = jst["ot"], jst["pgd"]
                    if jst["prev"] is not None:
                        _emit_pv(nc, ot, vaug, jst["prev"], ntiles)
                    for j in range(4):
                        nc.tensor.matmul(
                            ot[:], vaug[:, 4 * mb + j, :], pgd[:, j, :],
                            start=(nsub == 0 and j == 0), stop=(j == 3),
                            skip_group_check=True,
                        )
                    # epilogue: transpose + normalize + store
                    ot_sb = epi_pool.tile([D + 1, QB], F32, tag="ot_sb")
                    nc.vector.tensor_copy(out=ot_sb[:], in_=ot[:])
                    tpo = ot_pool.tile([P, 4, D + 1], F32, tag="ot", name="tpo")
                    for j in range(4):
                        nc.tensor.transpose(
                            tpo[:, j, :],
                            ot_sb[:, j * P : (j + 1) * P],
                            identity[: D + 1, : D + 1],
                        )
                    linv = epi_pool.tile([P, 4], F32, tag="linv")
                    nc.vector.reciprocal(linv[:], tpo[:, :, D])
                    o_sb = epi_pool.tile([P, 4, D], F32, tag="o_sb")
                    for j in range(4):
                        nc.vector.tensor_scalar_mul(
                            o_sb[:, j, :], tpo[:, j, :D], linv[:, j : j + 1]
                        )
                    nc.sync.dma_start(
                        out[h, mb * QB : (mb + 1) * QB, :].rearrange(
                            "(j p) d -> p j d", p=P
                        ),
                        o_sb[:],
                    )

                chunks = [sub_group(s) for s in range(0, nsub, G)]
                chunks += [diag_group(0), diag_group(1), pv_epilogue]
                return chunks

            # ---- software-pipelined emission: depth-2 job interleave ----
            for c in setup_chunks(0):
                c()
            pending = []          # next head's setup chunks, dripped in
            jobs = [(h, mb) for h in range(HPC) for mb in range(MBS)]
            active = []           # up to 2 jobs' chunk queues
            ji = 0
            drip = 0
            while active or ji < len(jobs):
                while len(active) < 2 and ji < len(jobs):
                    h, mb = jobs[ji]
                    if mb == 0 and pending:
                        # head h's setup must be fully emitted before its
                        # first job
                        for c in pending:
                            c()
                        pending = []
                    if mb == 0 and h + 1 < HPC:
                        pending = list(setup_chunks(h + 1))
                    active.append(job_chunks(h, mb))
                    ji += 1
                for q_ in list(active):
                    q_.pop(0)()
                    drip += 1
                    if drip % 3 == 0 and pending:
                        pending.pop(0)()
                active = [q_ for q_ in active if q_]
            for c in pending:
                c()

    nc.compile()
    return nc


def _emit_pv(nc, ot, vaug, group, ntiles):
    pg, tiles = group
    for j, nt in enumerate(tiles):
        nc.tensor.matmul(
            ot[:],
            vaug[:, nt, :],
            pg[:, j, :],
            start=(nt == 0),
            stop=(nt == ntiles - 1),
            skip_group_check=True,
        )


_NC = None


def _get_nc():
    global _NC
    if _NC is None:
        _NC = build_nc()
    return _NC


def kernel(q: np.ndarray, k: np.ndarray, v: np.ndarray) -> np.ndarray:
    from concourse.bass_utils import run_bass_kernel_spmd

    nc = _get_nc()
    qf = np.ascontiguousarray(np.asarray(q, dtype=np.float32).reshape(B * H, N, D))
    kf = np.ascontiguousarray(np.asarray(k, dtype=np.float32).reshape(B * H, N, D))
    vf = np.ascontiguousarray(np.asarray(v, dtype=np.float32).reshape(B * H, N, D))
    in_maps = [
        {
            "q": np.ascontiguousarray(qf[c * HPC : (c + 1) * HPC]),
            "k": np.ascontiguousarray(kf[c * HPC : (c + 1) * HPC]),
            "v": np.ascontiguousarray(vf[c * HPC : (c + 1) * HPC]),
        }
        for c in range(NCORES)
    ]
    res = run_bass_kernel_spmd(nc, in_maps, core_ids=list(range(NCORES)))
    outs = [res.results[c]["out"] for c in range(NCORES)]
    return np.concatenate(outs, axis=0).reshape(B, H, N, D)


if __name__ == "__main__":
    rng = np.random.default_rng(0)
    qq = rng.standard_normal((B, H, N, D), dtype=np.float32)
    kk = rng.standard_normal((B, H, N, D), dtype=np.float32)
    vv = rng.standard_normal((B, H, N, D), dtype=np.float32)
    o = kernel(q=qq, k=kk, v=vv)
    print("kernel ran, out shape", o.shape, "finite:", np.isfinite(o).all())



# revision 10
# speedup vs baseline: 1.1245x; 1.1245x over previous
"""RBF-kernel causal attention on 8 Trainium2 NeuronCores.

B=2, H=16, N=2048, D=64. Shards the 32 (b,h) attention instances across 8
cores (4 heads per core). Math notes:

  logits = -relu(||q-k||^2)/sqrt(D); relu is a no-op (||q-k||^2 >= 0 up to
  rounding), and softmax is invariant to per-query offsets, so
      softmax_n(-(qsq_m + ksq_n - 2 qk)/8) == softmax_n(qk/4 - ksq_n/8)
  The per-key term is folded into the QK matmul itself: k tiles are extended
  with a 65th row holding ksq_n and q tiles with a 65th row holding -0.5, so
  the 65-partition contraction directly yields qk - ksq/2, and
      P = exp(0.25 * (K Q^T - ksq/2))        in a [key, query] layout.
  V is extended with a ones column (vaug, bf16); the PV step runs P tiles as
  the STATIONARY operand ([128 key, 128 query] bf16) against vaug as the
  moving operand ([128 key, 65]), accumulating O[query, d] | l[query] directly
  in natural orientation -- no output transpose, and only 65 moving rows per
  key tile.  Final output O[m,d] = acc[m,d] / l[m].

Emission is manually software-pipelined: head h+1's setup chunks (transposes,
ksq, vaug build) are interleaved between head h's query blocks so the tile
scheduler (limited lookahead) can overlap them.
"""

import sys

if "/opt/trn_rl_repo" not in sys.path:
    sys.path.insert(0, "/opt/trn_rl_repo")

import numpy as np

import concourse.bacc as bacc
import concourse.mybir as mybir
import concourse.tile as tile
from concourse.masks import make_identity

B, H, N, D = 2, 16, 2048, 64
NCORES = 8
HPC = (B * H) // NCORES  # heads per core = 4
P = 128                  # partitions
NT = N // P              # key tiles per head = 16
QB = 512                 # query block = 4 query sub-tiles of 128
MBS = N // QB            # query blocks per head = 4
G = 2                    # key tiles per exp/ACT group (2 PSUM banks)
DE = D + 1               # extended depth (65): ksq row / ones column

F32 = mybir.dt.float32
# float32r = relaxed-precision fp32 matmul (1 cycle/row at moving dim >= 256
# instead of 4 for float32); bit-identical data to f32.
MM_DT = mybir.dt.float32r
BF16 = mybir.dt.bfloat16


def build_nc():
    nc = bacc.Bacc("TRN2", target_bir_lowering=False, debug=False)
    q = nc.dram_tensor("q", [HPC, N, D], F32, kind="ExternalInput")
    k = nc.dram_tensor("k", [HPC, N, D], F32, kind="ExternalInput")
    v = nc.dram_tensor("v", [HPC, N, D], F32, kind="ExternalInput")
    out = nc.dram_tensor("out", [HPC, N, D], F32, kind="ExternalOutput")

    with tile.TileContext(nc) as tc:
        with (
            tc.tile_pool(name="const", bufs=1) as const_pool,
            tc.tile_pool(name="loads", bufs=1) as load_pool,
            tc.tile_pool(name="head", bufs=2) as head_pool,
            tc.tile_pool(name="work", bufs=2) as work_pool,
            tc.tile_pool(name="p", bufs=4) as p_pool,
            tc.tile_pool(name="epi", bufs=3) as epi_pool,
            tc.tile_pool(name="st", bufs=3, space="PSUM") as st_pool,
            tc.tile_pool(name="otp", bufs=2, space="PSUM") as ot_pool,
        ):
            identity = const_pool.tile([P, P], F32)
            make_identity(nc, identity)
            # multiplicative causal masks for odd diagonal tiles (jj=1,3):
            # mask[jj][n, m] = 1.0 if m - n - 128*jj >= 0 else 0.0
            dmask = {}
            for jj in (1, 3):
                mk = const_pool.tile([P, QB], BF16, tag=f"dmask{jj}", name="mk")
                nc.gpsimd.memset(mk[:], 1.0)
                nc.gpsimd.affine_select(
                    out=mk[:], in_=mk[:],
                    compare_op=mybir.AluOpType.is_ge, fill=0.0,
                    base=-P * jj, pattern=[[1, QB]], channel_multiplier=-1,
                )
                dmask[jj] = mk

            # prefetch every head's inputs up front: no-wait DMAs stream in
            # the background while compute proceeds.  k/q land in the low 64
            # columns of 65-wide extended tiles (col 64 is filled on-chip).
            kexts, qexts, vtmps = [], [], []
            for h in range(HPC):
                kext = load_pool.tile([P, NT, DE], F32, tag=f"kext{h}")
                qext = load_pool.tile([P, NT, DE], F32, tag=f"qext{h}")
                vtmp = load_pool.tile([P, NT, D], F32, tag=f"vtmp{h}")
                kq = k[h].rearrange("(t p) d -> p t d", p=P)
                qq = q[h].rearrange("(t p) d -> p t d", p=P)
                vq = v[h].rearrange("(t p) d -> p t d", p=P)
                nch = 4 if h == 0 else 1
                w_ = NT // nch
                for c in range(nch):
                    ts = slice(w_ * c, w_ * c + w_)
                    nc.sync.dma_start(kext[:, ts, :D], kq[:, ts, :])
                    nc.sync.dma_start(qext[:, ts, :D], qq[:, ts, :])
                    nc.sync.dma_start(vtmp[:, ts, :], vq[:, ts, :])
                kexts.append(kext)
                qexts.append(qext)
                vtmps.append(vtmp)

            heads = [{} for _ in range(HPC)]

            def setup_chunks(h):
                """Emission chunks for head h's setup, in dependency order."""
                st = heads[h]
                kext, qext, vtmp = kexts[h], qexts[h], vtmps[h]

                def allocs():
                    # kt/qt: transposed 65-row operands [d(+ksq | -0.5), keys]
                    st["kt"] = head_pool.tile([DE, NT, P], MM_DT, tag="kt", name="kt")
                    st["qt"] = head_pool.tile([DE, NT, P], MM_DT, tag="qt", name="qt")
                    st["vaug"] = head_pool.tile(
                        [P, NT, DE], BF16, tag="vaug", name="vaug"
                    )
                    # constant 65th rows: q gets -0.5 (so ksq*q65 = -ksq/2),
                    # vaug gets the ones column for the softmax denominator
                    nc.gpsimd.memset(qext[:, :, D], -0.5)
                    nc.gpsimd.memset(st["vaug"][:, :, D], 1.0)

                def prep_chunk(c, nt4=4):
                    # per-quarter ksq + vaug build: runs as soon as that
                    # quarter of k and v has landed (head 0 only; later heads
                    # build whole-head to save per-instruction overhead)
                    def run():
                        ts = slice(4 * c, 4 * c + nt4)
                        ktmp = work_pool.tile([P, nt4, D], F32, tag="ktmp")
                        nc.vector.tensor_mul(
                            out=ktmp[:, :nt4], in0=kext[:, ts, :D], in1=kext[:, ts, :D]
                        )
                        nc.vector.tensor_reduce(
                            kext[:, ts, D], ktmp[:, :nt4],
                            axis=mybir.AxisListType.X, op=mybir.AluOpType.add,
                        )
                        nc.gpsimd.tensor_copy(
                            out=st["vaug"][:, ts, :D], in_=vtmp[:, ts, :]
                        )

                    return run

                def ktr_group(g):
                    # 4 transposes: [128 n, 65] -> [65, 128 n]
                    def run():
                        dst = st["kt"]
                        tp = st_pool.tile([DE, 4, P], F32, tag="stg", name="tp")
                        for j in range(4):
                            t = 4 * g + j
                            nc.tensor.transpose(
                                tp[:, j, :], kext[:, t, :], identity[:]
                            )
                        nc.vector.tensor_copy(
                            out=dst[:, 4 * g : 4 * g + 4, :], in_=tp[:]
                        )

                    return run

                def qtr_group(g):
                    def run():
                        dst = st["qt"]
                        tp = st_pool.tile([DE, 4, P], F32, tag="stg", name="tp")
                        for j in range(4):
                            t = 4 * g + j
                            nc.tensor.transpose(
                                tp[:, j, :], qext[:, t, :], identity[:]
                            )
                        nc.vector.tensor_copy(
                            out=dst[:, 4 * g : 4 * g + 4, :], in_=tp[:]
                        )

                    return run

                # query block mb needs kt tiles <= 4mb+3, qt group mb, vaug
                # quarter mb; yield in dependency order
                yield allocs
                if h == 0:
                    yield prep_chunk(0)
                    yield ktr_group(0)
                    yield qtr_group(0)
                    yield prep_chunk(1)
                    yield ktr_group(1)
                    yield qtr_group(1)
                    yield prep_chunk(2)
                    yield ktr_group(2)
                    yield qtr_group(2)
                    yield prep_chunk(3)
                    yield ktr_group(3)
                    yield qtr_group(3)
                else:
                    yield prep_chunk(0, NT)
                    yield ktr_group(0)
                    yield qtr_group(0)
                    yield ktr_group(1)
                    yield qtr_group(1)
                    yield ktr_group(2)
                    yield qtr_group(2)
                    yield ktr_group(3)
                    yield qtr_group(3)

            def job_chunks(h, mb):
                """Chunks of one (head, query-block) job, for interleaving."""
                kt, qt, vaug = heads[h]["kt"], heads[h]["qt"], heads[h]["vaug"]
                nsub = 4 * mb          # sub-diagonal key tiles
                rhs_q = qt[:, 4 * mb : 4 * mb + 4, :]    # [65, 512]
                jst = {"prev": None, "ot": None}

                def sub_group(s):
                    def run():
                        if jst["ot"] is None:
                            jst["ot"] = ot_pool.tile(
                                [P, 4, P], F32, tag="ot", name="ot"
                            )
                        stg = st_pool.tile([P, G, QB], F32, tag="stg")
                        for i in range(G):
                            nc.tensor.matmul(
                                stg[:, i, :], kt[:, s + i, :], rhs_q,
                                start=True, stop=True, skip_group_check=True,
                            )
                        pg = p_pool.tile([P, G, QB], BF16, tag="pg")
                        nc.scalar.activation(
                            pg[:], stg[:],
                            mybir.ActivationFunctionType.Exp, scale=0.25,
                        )
                        if jst["prev"] is not None:
                            _emit_pv(nc, jst["ot"], vaug, jst["prev"])
                        jst["prev"] = (pg, [s, s + 1])

                    return run

                def diag_group(a):
                    def run():
                        if jst["ot"] is None:
                            jst["ot"] = ot_pool.tile(
                                [P, 4, P], F32, tag="ot", name="ot"
                            )
                        if a == 0:
                            jst["pgd"] = p_pool.tile(
                                [P, 4, QB], BF16, tag="pgd", name="pgd"
                            )
                            # the masked-out region read by the mask ops below
                            # is never written by the exp; zero it so those
                            # reads are initialised
                            nc.gpsimd.memset(jst["pgd"][:, 2:4, : 2 * P], 0.0)
                        pgd = jst["pgd"]
                        # columns m < 256a of tiles (2a, 2a+1) are fully
                        # masked: skip their QK matmul + exp; affine_select /
                        # dmask below zero-fill that (otherwise stale) region.
                        c0 = 2 * P * a
                        stg = st_pool.tile([P, G, QB], F32, tag="stg")
                        for i in range(G):
                            nc.tensor.matmul(
                                stg[:, i, c0:],
                                kt[:, 4 * mb + 2 * a + i, :],
                                qt[:, 4 * mb + 2 * a : 4 * mb + 4, :],
                                start=True, stop=True, skip_group_check=True,
                            )
                        nc.scalar.activation(
                            pgd[:, 2 * a : 2 * a + 2, c0:], stg[:, :, c0:],
                            mybir.ActivationFunctionType.Exp, scale=0.25,
                        )
                        # keep pgd[n, jj, m] iff m - n - 128 jj >= 0; the
                        # even tile masks on Pool (affine_select), the odd
                        # ones concurrently on DVE (mul by a const mask)
                        jj = 2 * a
                        nc.gpsimd.affine_select(
                            out=pgd[:, jj, :], in_=pgd[:, jj, :],
                            compare_op=mybir.AluOpType.is_ge, fill=0.0,
                            base=-P * jj, pattern=[[1, QB]],
                            channel_multiplier=-1,
                        )
                        nc.vector.tensor_mul(
                            out=pgd[:, jj + 1, :],
                            in0=pgd[:, jj + 1, :],
                            in1=dmask[jj + 1][:],
                        )

                    return run

                def pv_epilogue():
                    ot, pgd = jst["ot"], jst["pgd"]
                    if jst["prev"] is not None:
                        _emit_pv(nc, ot, vaug, jst["prev"])
                    # diagonal PV: query sub-tile j only takes contributions
                    # from diag tiles jj <= j (the rest are fully masked)
                    # PSUM zero-region semantics: start=True resets the whole
                    # 2KB bank, so only the very FIRST matmul into the ot bank
                    # may set it; later sub-tiles' first writes land on
                    # pending-zero bytes and overwrite (not accumulate).
                    for jj in range(4):
                        for j in range(jj, 4):
                            nc.tensor.matmul(
                                ot[:, j, :DE],
                                pgd[:, jj, j * P : (j + 1) * P],
                                vaug[:, 4 * mb + jj, :],
                                start=(nsub == 0 and jj == 0 and j == 0),
                                stop=(jj == j),
                                skip_group_check=True,
                            )
                    # epilogue: normalize + store (output is already in
                    # [query, d] orientation -- no transpose needed)
                    linv = epi_pool.tile([P, 4], F32, tag="linv")
                    nc.vector.reciprocal(linv[:], ot[:, :, D])
                    o_sb = epi_pool.tile([P, 4, D], F32, tag="o_sb")
                    nc.vector.tensor_mul(
                        out=o_sb[:],
                        in0=ot[:, :, :D],
                        in1=linv[:, :, None].to_broadcast((P, 4, D)),
                    )
                    nc.sync.dma_start(
                        out[h, mb * QB : (mb + 1) * QB, :].rearrange(
                            "(j p) d -> p j d", p=P
                        ),
                        o_sb[:],
                    )

                chunks = [sub_group(s) for s in range(0, nsub, G)]
                chunks += [diag_group(0), diag_group(1), pv_epilogue]
                return chunks

            # ---- software-pipelined emission: depth-2 job interleave ----
            for c in setup_chunks(0):
                c()
            pending = []          # next head's setup chunks, dripped in
            jobs = [(h, mb) for h in range(HPC) for mb in range(MBS)]
            active = []           # up to 2 jobs' chunk queues
            ji = 0
            drip = 0
            while active or ji < len(jobs):
                while len(active) < 2 and ji < len(jobs):
                    h, mb = jobs[ji]
                    if mb == 0 and pending:
                        # head h's setup must be fully emitted before its
                        # first job
                        for c in pending:
                            c()
                        pending = []
                    if mb == 0 and h + 1 < HPC:
                        pending = list(setup_chunks(h + 1))
                    active.append(job_chunks(h, mb))
                    ji += 1
                for q_ in list(active):
                    q_.pop(0)()
                    drip += 1
                    if drip % 3 == 0 and pending:
                        pending.pop(0)()
                active = [q_ for q_ in active if q_]
            for c in pending:
                c()

    nc.compile()
    return nc


def _emit_pv(nc, ot, vaug, group):
    """PV for a full (unmasked) pair of key tiles: pg tiles are stationary
    [128 key, 128 query] operands, vaug [128 key, 65] moves."""
    pg, tiles = group
    for i, nt in enumerate(tiles):
        for j in range(4):
            # start only on the bank's very first matmul (see pv_epilogue)
            nc.tensor.matmul(
                ot[:, j, : D + 1],
                pg[:, i, j * P : (j + 1) * P],
                vaug[:, nt, :],
                start=(nt == 0 and j == 0),
                stop=False,
                skip_group_check=True,
            )


_NC = None


def _get_nc():
    global _NC
    if _NC is None:
        _NC = build_nc()
    return _NC


def kernel(q: np.ndarray, k: np.ndarray, v: np.ndarray) -> np.ndarray:
    from concourse.bass_utils import run_bass_kernel_spmd

    nc = _get_nc()
    qf = np.ascontiguousarray(np.asarray(q, dtype=np.float32).reshape(B * H, N, D))
    kf = np.ascontiguousarray(np.asarray(k, dtype=np.float32).reshape(B * H, N, D))
    vf = np.ascontiguousarray(np.asarray(v, dtype=np.float32).reshape(B * H, N, D))
    in_maps = [
        {
            "q": np.ascontiguousarray(qf[c * HPC : (c + 1) * HPC]),
            "k": np.ascontiguousarray(kf[c * HPC : (c + 1) * HPC]),
            "v": np.ascontiguousarray(vf[c * HPC : (c + 1) * HPC]),
        }
        for c in range(NCORES)
    ]
    res = run_bass_kernel_spmd(nc, in_maps, core_ids=list(range(NCORES)))
    outs = [res.results[c]["out"] for c in range(NCORES)]
    return np.concatenate(outs, axis=0).reshape(B, H, N, D)


if __name__ == "__main__":
    rng = np.random.default_rng(0)
    qq = rng.standard_normal((B, H, N, D), dtype=np.float32)
    kk = rng.standard_normal((B, H, N, D), dtype=np.float32)
    vv = rng.standard_normal((B, H, N, D), dtype=np.float32)
    o = kernel(q=qq, k=kk, v=vv)
    print("kernel ran, out shape", o.shape, "finite:", np.isfinite(o).all())


# revision 44
# speedup vs baseline: 1.2180x; 1.0832x over previous
"""RBF-kernel causal attention on 8 Trainium2 NeuronCores.

B=2, H=16, N=2048, D=64. Shards the 32 (b,h) attention instances across 8
cores (4 heads per core). Math notes:

  logits = -relu(||q-k||^2)/sqrt(D); relu is a no-op (||q-k||^2 >= 0 up to
  rounding), and softmax is invariant to per-query offsets, so
      softmax_n(-(qsq_m + ksq_n - 2 qk)/8) == softmax_n(qk/4 - ksq_n/8)
  The per-key term is folded into the QK matmul itself: k tiles are extended
  with a 65th row holding ksq_n and q tiles with a 65th row holding -0.5, so
  the 65-partition contraction directly yields qk - ksq/2, and
      P = exp(0.25 * (K Q^T - ksq/2))        in a [key, query] layout.
  V is extended with a ones column (vaug, bf16); the PV step runs P tiles as
  the STATIONARY operand ([128 key, 128 query] bf16) against vaug as the
  moving operand ([128 key, 65]), accumulating O[query, d] | l[query] directly
  in natural orientation -- no output transpose, and only 65 moving rows per
  key tile.  Final output O[m,d] = acc[m,d] / l[m].

Emission is manually software-pipelined: head h+1's setup chunks (transposes,
ksq, vaug build) are interleaved between head h's query blocks so the tile
scheduler (limited lookahead) can overlap them.
"""

import sys

if "/opt/trn_rl_repo" not in sys.path:
    sys.path.insert(0, "/opt/trn_rl_repo")

import numpy as np

import concourse.bacc as bacc
import concourse.mybir as mybir
import concourse.tile as tile
B, H, N, D = 2, 16, 2048, 64
NCORES = 8
HPC = (B * H) // NCORES  # heads per core = 4
P = 128                  # partitions
NT = N // P              # key tiles per head = 16
QB = 512                 # query block = 4 query sub-tiles of 128
MBS = N // QB            # query blocks per head = 4
G = 2                    # key tiles per exp/ACT group (2 PSUM banks)
DE = D + 1               # extended depth (65): ksq row / ones column

F32 = mybir.dt.float32
# float32r = relaxed-precision fp32 matmul (1 cycle/row at moving dim >= 256
# instead of 4 for float32); bit-identical data to f32.
MM_DT = mybir.dt.float32r
BF16 = mybir.dt.bfloat16


def build_nc():
    nc = bacc.Bacc("TRN2", target_bir_lowering=False, debug=False)
    q = nc.dram_tensor("q", [HPC, N, D], F32, kind="ExternalInput")
    k = nc.dram_tensor("k", [HPC, N, D], F32, kind="ExternalInput")
    v = nc.dram_tensor("v", [HPC, N, D], F32, kind="ExternalInput")
    out = nc.dram_tensor("out", [HPC, N, D], F32, kind="ExternalOutput")

    with tile.TileContext(nc) as tc:
        with (
            tc.tile_pool(name="const", bufs=1) as const_pool,
            tc.tile_pool(name="loads", bufs=1) as load_pool,
            tc.tile_pool(name="head", bufs=2) as head_pool,
            tc.tile_pool(name="work", bufs=4) as work_pool,
            tc.tile_pool(name="p", bufs=4) as p_pool,
            tc.tile_pool(name="epi", bufs=6) as epi_pool,
            tc.tile_pool(name="st", bufs=2, space="PSUM") as st_pool,
            tc.tile_pool(name="tpp", bufs=2, space="PSUM") as tp_pool,
            tc.tile_pool(name="otp", bufs=2, space="PSUM") as ot_pool,
        ):
            # identity on DVE (Pool is busy with other startup memsets) and a
            # warm-up transpose right behind it: the PE clock ramps to full
            # speed only after ~3us of busy history, so starting the ramp at
            # ~1us makes the real transposes and first QKs run 2x faster
            identity = const_pool.tile([P, P], F32)
            nc.vector.memset(identity[:], 0.0)
            nc.gpsimd.affine_select(
                out=identity[:], in_=identity[:],
                compare_op=mybir.AluOpType.not_equal, fill=1.0,
                base=0, pattern=[[-1, P]], channel_multiplier=1,
            )
            wtp = tp_pool.tile([DE, 4, P], F32, tag="tp", name="wtp")
            nc.tensor.transpose(wtp[:, 0, :], identity[:, :DE], identity[:])
            # multiplicative causal masks for the diagonal tile pairs:
            # dmask[a][n, jj-2a, m] = 1.0 if m - n - 128*jj >= 0 else 0.0,
            # applied to both tiles of a diag exp group in one DVE mul.
            # Built lazily (as a setup chunk) so the ~4us of Pool work stays
            # off the startup critical path (ksq -> transposes -> first QK).
            dmask = {}

            def build_dmasks():
                for a in (0, 1):
                    mk = const_pool.tile([P, 2, QB], BF16, tag=f"dmask{a}", name="mk")
                    nc.gpsimd.memset(mk[:], 1.0)
                    for i in (0, 1):
                        jj = 2 * a + i
                        nc.gpsimd.affine_select(
                            out=mk[:, i, :], in_=mk[:, i, :],
                            compare_op=mybir.AluOpType.is_ge, fill=0.0,
                            base=-P * jj, pattern=[[1, QB]], channel_multiplier=-1,
                        )
                    dmask[a] = mk

            # prefetch every head's inputs up front: no-wait DMAs stream in
            # the background while compute proceeds.  k/q land in the low 64
            # columns of 65-wide extended tiles (col 64 is filled on-chip).
            kexts, qexts, vtmps = [], [], []
            for h in range(HPC):
                kext = load_pool.tile([P, NT, DE], F32, tag=f"kext{h}")
                qext = load_pool.tile([P, NT, DE], F32, tag=f"qext{h}")
                vtmp = load_pool.tile([P, NT, D], F32, tag=f"vtmp{h}")
                kq = k[h].rearrange("(t p) d -> p t d", p=P)
                qq = q[h].rearrange("(t p) d -> p t d", p=P)
                vq = v[h].rearrange("(t p) d -> p t d", p=P)
                if h == 0:
                    # quarter-granular, k/q prioritized so the first
                    # transposes can start after ~2 DMAs, v one quarter behind
                    order = [("k", 0), ("q", 0), ("k", 1), ("q", 1), ("v", 0),
                             ("k", 2), ("q", 2), ("v", 1), ("k", 3), ("q", 3),
                             ("v", 2), ("v", 3)]
                    for which, c in order:
                        ts = slice(4 * c, 4 * c + 4)
                        if which == "k":
                            nc.sync.dma_start(kext[:, ts, :D], kq[:, ts, :])
                        elif which == "q":
                            nc.sync.dma_start(qext[:, ts, :D], qq[:, ts, :])
                        else:
                            nc.sync.dma_start(vtmp[:, ts, :], vq[:, ts, :])
                # heads >= 1 issue their DMAs lazily from prep_chunk: a
                # whole-head DMA would monopolize the DMA engines for ~3us
                # and starve the per-job output stores queued behind it
                kexts.append(kext)
                qexts.append(qext)
                vtmps.append(vtmp)

            heads = [{} for _ in range(HPC)]

            def setup_chunks(h):
                """Emission chunks for head h's setup, in dependency order."""
                st = heads[h]
                kext, qext, vtmp = kexts[h], qexts[h], vtmps[h]

                def allocs():
                    # kt/qt: transposed 65-row operands [d(+ksq | -0.5), keys]
                    st["kt"] = head_pool.tile([DE, NT, P], MM_DT, tag="kt", name="kt")
                    st["qt"] = head_pool.tile([DE, NT, P], MM_DT, tag="qt", name="qt")
                    st["vaug"] = head_pool.tile(
                        [P, NT, DE], BF16, tag="vaug", name="vaug"
                    )
                    # constant 65th rows: q gets -0.5 (so ksq*q65 = -ksq/2),
                    # vaug gets the ones column for the softmax denominator
                    nc.gpsimd.memset(qext[:, :, D], -0.5)
                    nc.gpsimd.memset(st["vaug"][:, :, D], 1.0)

                def prep_chunk(c):
                    # per-quarter ksq + vaug build, square on Pool (DVE stays
                    # free for the per-job mask/copy work): runs as soon as
                    # that quarter of k and v has landed.  Head 0 quarter 0
                    # squares on DVE instead -- it gates the whole startup and
                    # Pool is busy building masks then.
                    sq_eng = nc.vector if h == 0 else nc.gpsimd

                    def run():
                        ts = slice(4 * c, 4 * c + 4)
                        if h > 0:
                            # lazy quarter loads (see prefetch note above)
                            kq = k[h].rearrange("(t p) d -> p t d", p=P)
                            qq = q[h].rearrange("(t p) d -> p t d", p=P)
                            vq = v[h].rearrange("(t p) d -> p t d", p=P)
                            nc.sync.dma_start(kext[:, ts, :D], kq[:, ts, :])
                            nc.sync.dma_start(qext[:, ts, :D], qq[:, ts, :])
                            nc.sync.dma_start(vtmp[:, ts, :], vq[:, ts, :])
                        ktmp = work_pool.tile([P, 4, D], F32, tag="ktmp")
                        sq_eng.tensor_mul(
                            out=ktmp[:], in0=kext[:, ts, :D], in1=kext[:, ts, :D]
                        )
                        nc.vector.tensor_reduce(
                            kext[:, ts, D], ktmp[:],
                            axis=mybir.AxisListType.X, op=mybir.AluOpType.add,
                        )
                        nc.gpsimd.tensor_copy(
                            out=st["vaug"][:, ts, :D], in_=vtmp[:, ts, :]
                        )

                    return run

                def ktr_group(g):
                    # 4 transposes: [128 n, 65] -> [65, 128 n]
                    def run():
                        dst = st["kt"]
                        tp = tp_pool.tile([DE, 4, P], F32, tag="tp", name="tp")
                        for j in range(4):
                            t = 4 * g + j
                            nc.tensor.transpose(
                                tp[:, j, :], kext[:, t, :], identity[:]
                            )
                        nc.vector.tensor_copy(
                            out=dst[:, 4 * g : 4 * g + 4, :], in_=tp[:]
                        )

                    return run

                def qtr_group(g):
                    def run():
                        dst = st["qt"]
                        tp = tp_pool.tile([DE, 4, P], F32, tag="tp", name="tp")
                        for j in range(4):
                            t = 4 * g + j
                            nc.tensor.transpose(
                                tp[:, j, :], qext[:, t, :], identity[:]
                            )
                        nc.vector.tensor_copy(
                            out=dst[:, 4 * g : 4 * g + 4, :], in_=tp[:]
                        )

                    return run

                # query block mb needs kt tiles <= 4mb+3, qt group mb, vaug
                # quarter mb; yield in dependency order.  ksq (prep) gates
                # the k transposes, so preps lead their quarter's groups.
                yield allocs
                if h == 0:
                    for c in range(4):
                        yield prep_chunk(c)
                        yield ktr_group(c)
                        yield qtr_group(c)
                        if c == 0:
                            # after quarter 0 is rolling, build the const
                            # masks (Pool) before job (0,0)'s diag groups
                            yield build_dmasks
                else:
                    # all preps first: they are cheap, their DMA deps landed
                    # long ago, and everything else queues behind them in
                    # each engine's in-order stream
                    for c in range(4):
                        yield prep_chunk(c)
                    for c in range(4):
                        yield ktr_group(c)
                        yield qtr_group(c)

            def job_chunks(h, mb):
                """Chunks of one (head, query-block) job, for interleaving."""
                kt, qt, vaug = heads[h]["kt"], heads[h]["qt"], heads[h]["vaug"]
                nsub = 4 * mb          # sub-diagonal key tiles
                rhs_q = qt[:, 4 * mb : 4 * mb + 4, :]    # [65, 512]
                jst = {"prev": None, "ot": None}

                def sub_group(s):
                    def run():
                        if jst["ot"] is None:
                            jst["ot"] = ot_pool.tile(
                                [P, 4, P], F32, tag="ot", name="ot"
                            )
                        stg = st_pool.tile([P, G, QB], F32, tag="stg")
                        for i in range(G):
                            nc.tensor.matmul(
                                stg[:, i, :], kt[:, s + i, :], rhs_q,
                                start=True, stop=True, skip_group_check=True,
                            )
                        pg = p_pool.tile([P, G, QB], BF16, tag="pg")
                        nc.scalar.activation(
                            pg[:], stg[:],
                            mybir.ActivationFunctionType.Exp, scale=0.25,
                        )
                        if jst["prev"] is not None:
                            _emit_pv(nc, jst["ot"], vaug, jst["prev"])
                        jst["prev"] = (pg, [s, s + 1])

                    return run

                def diag_group(a):
                    def run():
                        if jst["ot"] is None:
                            jst["ot"] = ot_pool.tile(
                                [P, 4, P], F32, tag="ot", name="ot"
                            )
                        if a == 0:
                            jst["pgd"] = p_pool.tile(
                                [P, 4, QB], BF16, tag="pgd", name="pgd"
                            )
                            # the masked-out region read by the mask ops below
                            # is never written by the exp; zero it so those
                            # reads are initialised
                            nc.gpsimd.memset(jst["pgd"][:, 2:4, : 2 * P], 0.0)
                        pgd = jst["pgd"]
                        # columns m < 256a of tiles (2a, 2a+1) are fully
                        # masked: skip their QK matmul + exp; affine_select /
                        # dmask below zero-fill that (otherwise stale) region.
                        c0 = 2 * P * a
                        stg = st_pool.tile([P, G, QB], F32, tag="stg")
                        for i in range(G):
                            nc.tensor.matmul(
                                stg[:, i, c0:],
                                kt[:, 4 * mb + 2 * a + i, :],
                                qt[:, 4 * mb + 2 * a : 4 * mb + 4, :],
                                start=True, stop=True, skip_group_check=True,
                            )
                        nc.scalar.activation(
                            pgd[:, 2 * a : 2 * a + 2, c0:], stg[:, :, c0:],
                            mybir.ActivationFunctionType.Exp, scale=0.25,
                        )
                        # keep pgd[n, jj, m] iff m - n - 128 jj >= 0: one DVE
                        # mul by the const mask pair (2-byte dtype, 2x mode)
                        nc.vector.tensor_mul(
                            out=pgd[:, 2 * a : 2 * a + 2, :],
                            in0=pgd[:, 2 * a : 2 * a + 2, :],
                            in1=dmask[a][:],
                        )

                    return run

                def pv_epilogue():
                    ot, pgd = jst["ot"], jst["pgd"]
                    if jst["prev"] is not None:
                        _emit_pv(nc, ot, vaug, jst["prev"])
                    # diagonal PV: query sub-tile j only takes contributions
                    # from diag tiles jj <= j (the rest are fully masked)
                    # PSUM zero-region semantics: start=True resets the whole
                    # 2KB bank, so only the very FIRST matmul into the ot bank
                    # may set it; later sub-tiles' first writes land on
                    # pending-zero bytes and overwrite (not accumulate).
                    for jj in range(4):
                        for j in range(jj, 4):
                            nc.tensor.matmul(
                                ot[:, j, :DE],
                                pgd[:, jj, j * P : (j + 1) * P],
                                vaug[:, 4 * mb + jj, :],
                                start=(nsub == 0 and jj == 0 and j == 0),
                                stop=(jj == j),
                                skip_group_check=True,
                            )
                    # epilogue: normalize + store (output is already in
                    # [query, d] orientation -- no transpose needed)
                    linv = epi_pool.tile([P, 4], F32, tag="linv")
                    nc.vector.reciprocal(linv[:], ot[:, :, D])
                    o_sb = epi_pool.tile([P, 4, D], F32, tag="o_sb")
                    nc.vector.tensor_mul(
                        out=o_sb[:],
                        in0=ot[:, :, :D],
                        in1=linv[:, :, None].to_broadcast((P, 4, D)),
                    )
                    nc.sync.dma_start(
                        out[h, mb * QB : (mb + 1) * QB, :].rearrange(
                            "(j p) d -> p j d", p=P
                        ),
                        o_sb[:],
                    )

                chunks = [sub_group(s) for s in range(0, nsub, G)]
                chunks += [diag_group(0), diag_group(1), pv_epilogue]
                return chunks

            # ---- software-pipelined emission: depth-2 job interleave ----
            # head 0: emit only the first two quarters' setup up front, drip
            # the rest between job chunks so the first QK isn't queued behind
            # every transpose on PE.  Emission order defines dependencies, so
            # job (0,mb) must have its quarters' setup emitted first:
            # h0 chunk list is [allocs, prep0, ktr0, qtr0, dmasks,
            # (prep,ktr,qtr) x 3]; job (0,mb) needs the first 5+3*mb chunks.
            setup_q = {0: list(setup_chunks(0))}
            n0 = len(setup_q[0])
            for _ in range(8):
                setup_q[0].pop(0)()
            # later heads' setup chunks, dripped in ~1.5 heads ahead of use
            setup_q[1] = list(setup_chunks(1))

            def drip_one():
                for hh in sorted(setup_q):
                    if setup_q[hh]:
                        setup_q[hh].pop(0)()
                        return

            jobs = [(h, mb) for h in range(HPC) for mb in range(MBS)]
            active = []           # up to 2 jobs' chunk queues
            ji = 0
            while active or ji < len(jobs):
                while len(active) < 2 and ji < len(jobs):
                    h, mb = jobs[ji]
                    if h == 0:
                        while n0 - len(setup_q[0]) < 5 + 3 * mb:
                            setup_q[0].pop(0)()
                    else:
                        # head h's setup must be fully emitted before its
                        # first job
                        if mb == 0:
                            for c in setup_q.get(h, []):
                                c()
                            setup_q[h] = []
                    if mb == 0 and h + 1 < HPC and h + 1 not in setup_q:
                        setup_q[h + 1] = list(setup_chunks(h + 1))
                    active.append(job_chunks(h, mb))
                    ji += 1
                for q_ in list(active):
                    q_.pop(0)()
                    drip_one()
                    drip_one()
                active = [q_ for q_ in active if q_]
            for hh in sorted(setup_q):
                for c in setup_q[hh]:
                    c()

    nc.compile()
    return nc


def _emit_pv(nc, ot, vaug, group):
    """PV for a full (unmasked) pair of key tiles: pg tiles are stationary
    [128 key, 128 query] operands, vaug [128 key, 65] moves."""
    pg, tiles = group
    for i, nt in enumerate(tiles):
        for j in range(4):
            # start only on the bank's very first matmul (see pv_epilogue)
            nc.tensor.matmul(
                ot[:, j, : D + 1],
                pg[:, i, j * P : (j + 1) * P],
                vaug[:, nt, :],
                start=(nt == 0 and j == 0),
                stop=False,
                skip_group_check=True,
            )


_NC = None


def _get_nc():
    global _NC
    if _NC is None:
        _NC = build_nc()
    return _NC


def kernel(q: np.ndarray, k: np.ndarray, v: np.ndarray) -> np.ndarray:
    from concourse.bass_utils import run_bass_kernel_spmd

    nc = _get_nc()
    qf = np.ascontiguousarray(np.asarray(q, dtype=np.float32).reshape(B * H, N, D))
    kf = np.ascontiguousarray(np.asarray(k, dtype=np.float32).reshape(B * H, N, D))
    vf = np.ascontiguousarray(np.asarray(v, dtype=np.float32).reshape(B * H, N, D))
    in_maps = [
        {
            "q": np.ascontiguousarray(qf[c * HPC : (c + 1) * HPC]),
            "k": np.ascontiguousarray(kf[c * HPC : (c + 1) * HPC]),
            "v": np.ascontiguousarray(vf[c * HPC : (c + 1) * HPC]),
        }
        for c in range(NCORES)
    ]
    res = run_bass_kernel_spmd(nc, in_maps, core_ids=list(range(NCORES)))
    outs = [res.results[c]["out"] for c in range(NCORES)]
    return np.concatenate(outs, axis=0).reshape(B, H, N, D)


if __name__ == "__main__":
    rng = np.random.default_rng(0)
    qq = rng.standard_normal((B, H, N, D), dtype=np.float32)
    kk = rng.standard_normal((B, H, N, D), dtype=np.float32)
    vv = rng.standard_normal((B, H, N, D), dtype=np.float32)
    o = kernel(q=qq, k=kk, v=vv)
    print("kernel ran, out shape", o.shape, "finite:", np.isfinite(o).all())


# revision 46
# speedup vs baseline: 1.2263x; 1.0068x over previous
"""RBF-kernel causal attention on 8 Trainium2 NeuronCores.

B=2, H=16, N=2048, D=64. Shards the 32 (b,h) attention instances across 8
cores (4 heads per core). Math notes:

  logits = -relu(||q-k||^2)/sqrt(D); relu is a no-op (||q-k||^2 >= 0 up to
  rounding), and softmax is invariant to per-query offsets, so
      softmax_n(-(qsq_m + ksq_n - 2 qk)/8) == softmax_n(qk/4 - ksq_n/8)
  The per-key term is folded into the QK matmul itself: k tiles are extended
  with a 65th row holding ksq_n and q tiles with a 65th row holding -0.5, so
  the 65-partition contraction directly yields qk - ksq/2, and
      P = exp(0.25 * (K Q^T - ksq/2))        in a [key, query] layout.
  V is extended with a ones column (vaug, bf16); the PV step runs P tiles as
  the STATIONARY operand ([128 key, 128 query] bf16) against vaug as the
  moving operand ([128 key, 65]), accumulating O[query, d] | l[query] directly
  in natural orientation -- no output transpose, and only 65 moving rows per
  key tile.  Final output O[m,d] = acc[m,d] / l[m].

Emission is manually software-pipelined: head h+1's setup chunks (transposes,
ksq, vaug build) are interleaved between head h's query blocks so the tile
scheduler (limited lookahead) can overlap them.
"""

import sys

if "/opt/trn_rl_repo" not in sys.path:
    sys.path.insert(0, "/opt/trn_rl_repo")

import numpy as np

import concourse.bacc as bacc
import concourse.mybir as mybir
import concourse.tile as tile
B, H, N, D = 2, 16, 2048, 64
NCORES = 8
HPC = (B * H) // NCORES  # heads per core = 4
P = 128                  # partitions
NT = N // P              # key tiles per head = 16
QB = 512                 # query block = 4 query sub-tiles of 128
MBS = N // QB            # query blocks per head = 4
G = 2                    # key tiles per exp/ACT group (2 PSUM banks)
DE = D + 1               # extended depth (65): ksq row / ones column

F32 = mybir.dt.float32
# float32r = relaxed-precision fp32 matmul (1 cycle/row at moving dim >= 256
# instead of 4 for float32); bit-identical data to f32.
MM_DT = mybir.dt.float32r
BF16 = mybir.dt.bfloat16


def build_nc():
    nc = bacc.Bacc("TRN2", target_bir_lowering=False, debug=False)
    q = nc.dram_tensor("q", [HPC, N, D], F32, kind="ExternalInput")
    k = nc.dram_tensor("k", [HPC, N, D], F32, kind="ExternalInput")
    v = nc.dram_tensor("v", [HPC, N, D], F32, kind="ExternalInput")
    out = nc.dram_tensor("out", [HPC, N, D], F32, kind="ExternalOutput")

    with tile.TileContext(nc) as tc:
        with (
            tc.tile_pool(name="const", bufs=1) as const_pool,
            tc.tile_pool(name="loads", bufs=1) as load_pool,
            tc.tile_pool(name="head", bufs=2) as head_pool,
            tc.tile_pool(name="work", bufs=4) as work_pool,
            tc.tile_pool(name="p", bufs=4) as p_pool,
            tc.tile_pool(name="epi", bufs=6) as epi_pool,
            tc.tile_pool(name="st", bufs=2, space="PSUM") as st_pool,
            tc.tile_pool(name="tpp", bufs=2, space="PSUM") as tp_pool,
            tc.tile_pool(name="otp", bufs=2, space="PSUM") as ot_pool,
        ):
            # identity on DVE (Pool is busy with other startup memsets) and a
            # warm-up transpose right behind it: the PE clock ramps to full
            # speed only after ~3us of busy history, so starting the ramp at
            # ~1us makes the real transposes and first QKs run 2x faster
            identity = const_pool.tile([P, P], F32)
            nc.vector.memset(identity[:], 0.0)
            nc.gpsimd.affine_select(
                out=identity[:], in_=identity[:],
                compare_op=mybir.AluOpType.not_equal, fill=1.0,
                base=0, pattern=[[-1, P]], channel_multiplier=1,
            )
            wtp = tp_pool.tile([DE, 4, P], F32, tag="tp", name="wtp")
            nc.tensor.transpose(wtp[:, 0, :], identity[:, :DE], identity[:])
            # multiplicative causal masks for the diagonal tile pairs:
            # dmask[a][n, jj-2a, m] = 1.0 if m - n - 128*jj >= 0 else 0.0,
            # applied to both tiles of a diag exp group in one DVE mul.
            # Built lazily (as a setup chunk) so the ~4us of Pool work stays
            # off the startup critical path (ksq -> transposes -> first QK).
            dmask = {}

            def build_dmasks():
                for a in (0, 1):
                    mk = const_pool.tile([P, 2, QB], BF16, tag=f"dmask{a}", name="mk")
                    nc.gpsimd.memset(mk[:], 1.0)
                    for i in (0, 1):
                        jj = 2 * a + i
                        nc.gpsimd.affine_select(
                            out=mk[:, i, :], in_=mk[:, i, :],
                            compare_op=mybir.AluOpType.is_ge, fill=0.0,
                            base=-P * jj, pattern=[[1, QB]], channel_multiplier=-1,
                        )
                    dmask[a] = mk

            # prefetch every head's inputs up front: no-wait DMAs stream in
            # the background while compute proceeds.  k/q land in the low 64
            # columns of 65-wide extended tiles (col 64 is filled on-chip).
            kexts, qexts, vtmps = [], [], []
            for h in range(HPC):
                kext = load_pool.tile([P, NT, DE], F32, tag=f"kext{h}")
                qext = load_pool.tile([P, NT, DE], F32, tag=f"qext{h}")
                vtmp = load_pool.tile([P, NT, D], F32, tag=f"vtmp{h}")
                kq = k[h].rearrange("(t p) d -> p t d", p=P)
                qq = q[h].rearrange("(t p) d -> p t d", p=P)
                vq = v[h].rearrange("(t p) d -> p t d", p=P)
                if h == 0:
                    # quarter-granular, k/q prioritized so the first
                    # transposes can start after ~2 DMAs, v one quarter behind
                    order = [("k", 0), ("q", 0), ("k", 1), ("q", 1), ("v", 0),
                             ("k", 2), ("q", 2), ("v", 1), ("k", 3), ("q", 3),
                             ("v", 2), ("v", 3)]
                    for which, c in order:
                        ts = slice(4 * c, 4 * c + 4)
                        if which == "k":
                            nc.sync.dma_start(kext[:, ts, :D], kq[:, ts, :])
                        elif which == "q":
                            nc.sync.dma_start(qext[:, ts, :D], qq[:, ts, :])
                        else:
                            nc.sync.dma_start(vtmp[:, ts, :], vq[:, ts, :])
                # heads >= 1 issue their DMAs lazily from prep_chunk: a
                # whole-head DMA would monopolize the DMA engines for ~3us
                # and starve the per-job output stores queued behind it
                kexts.append(kext)
                qexts.append(qext)
                vtmps.append(vtmp)

            heads = [{} for _ in range(HPC)]

            def setup_chunks(h):
                """Emission chunks for head h's setup, in dependency order."""
                st = heads[h]
                kext, qext, vtmp = kexts[h], qexts[h], vtmps[h]

                def allocs():
                    # kt/qt: transposed 65-row operands [d(+ksq | -0.5), keys]
                    st["kt"] = head_pool.tile([DE, NT, P], MM_DT, tag="kt", name="kt")
                    st["qt"] = head_pool.tile([DE, NT, P], MM_DT, tag="qt", name="qt")
                    st["vaug"] = head_pool.tile(
                        [P, NT, DE], BF16, tag="vaug", name="vaug"
                    )
                    # constant 65th rows: q gets -0.5 (so ksq*q65 = -ksq/2),
                    # vaug gets the ones column for the softmax denominator
                    nc.gpsimd.memset(qext[:, :, D], -0.5)
                    nc.gpsimd.memset(st["vaug"][:, :, D], 1.0)

                def prep_chunk(c):
                    # per-quarter ksq + vaug build, square on Pool (DVE stays
                    # free for the per-job mask/copy work): runs as soon as
                    # that quarter of k and v has landed.  Head 0 quarter 0
                    # squares on DVE instead -- it gates the whole startup and
                    # Pool is busy building masks then.
                    sq_eng = nc.vector if h == 0 else nc.gpsimd

                    def run():
                        ts = slice(4 * c, 4 * c + 4)
                        if h > 0:
                            # lazy quarter loads (see prefetch note above)
                            kq = k[h].rearrange("(t p) d -> p t d", p=P)
                            qq = q[h].rearrange("(t p) d -> p t d", p=P)
                            vq = v[h].rearrange("(t p) d -> p t d", p=P)
                            nc.sync.dma_start(kext[:, ts, :D], kq[:, ts, :])
                            nc.sync.dma_start(qext[:, ts, :D], qq[:, ts, :])
                            nc.sync.dma_start(vtmp[:, ts, :], vq[:, ts, :])
                        ktmp = work_pool.tile([P, 4, D], F32, tag="ktmp")
                        sq_eng.tensor_mul(
                            out=ktmp[:], in0=kext[:, ts, :D], in1=kext[:, ts, :D]
                        )
                        nc.vector.tensor_reduce(
                            kext[:, ts, D], ktmp[:],
                            axis=mybir.AxisListType.X, op=mybir.AluOpType.add,
                        )
                        nc.gpsimd.tensor_copy(
                            out=st["vaug"][:, ts, :D], in_=vtmp[:, ts, :]
                        )

                    return run

                def ktr_group(g):
                    # 4 transposes: [128 n, 65] -> [65, 128 n]
                    def run():
                        dst = st["kt"]
                        tp = tp_pool.tile([DE, 4, P], F32, tag="tp", name="tp")
                        for j in range(4):
                            t = 4 * g + j
                            nc.tensor.transpose(
                                tp[:, j, :], kext[:, t, :], identity[:]
                            )
                        nc.vector.tensor_copy(
                            out=dst[:, 4 * g : 4 * g + 4, :], in_=tp[:]
                        )

                    return run

                def qtr_group(g):
                    def run():
                        dst = st["qt"]
                        tp = tp_pool.tile([DE, 4, P], F32, tag="tp", name="tp")
                        for j in range(4):
                            t = 4 * g + j
                            nc.tensor.transpose(
                                tp[:, j, :], qext[:, t, :], identity[:]
                            )
                        nc.vector.tensor_copy(
                            out=dst[:, 4 * g : 4 * g + 4, :], in_=tp[:]
                        )

                    return run

                # query block mb needs kt tiles <= 4mb+3, qt group mb, vaug
                # quarter mb; yield in dependency order.  ksq (prep) gates
                # the k transposes, so preps lead their quarter's groups.
                yield allocs
                if h == 0:
                    for c in range(4):
                        yield prep_chunk(c)
                        yield ktr_group(c)
                        yield qtr_group(c)
                        if c == 0:
                            # after quarter 0 is rolling, build the const
                            # masks (Pool) before job (0,0)'s diag groups
                            yield build_dmasks
                else:
                    # all preps first: they are cheap, their DMA deps landed
                    # long ago, and everything else queues behind them in
                    # each engine's in-order stream
                    for c in range(4):
                        yield prep_chunk(c)
                    for c in range(4):
                        yield ktr_group(c)
                        yield qtr_group(c)

            def job_chunks(h, mb):
                """Chunks of one (head, query-block) job, for interleaving."""
                kt, qt, vaug = heads[h]["kt"], heads[h]["qt"], heads[h]["vaug"]
                nsub = 4 * mb          # sub-diagonal key tiles
                rhs_q = qt[:, 4 * mb : 4 * mb + 4, :]    # [65, 512]
                jst = {"prev": None, "ot": None}

                def sub_group(s):
                    def run():
                        if jst["ot"] is None:
                            jst["ot"] = ot_pool.tile(
                                [P, 4, P], F32, tag="ot", name="ot"
                            )
                        stg = st_pool.tile([P, G, QB], F32, tag="stg")
                        for i in range(G):
                            nc.tensor.matmul(
                                stg[:, i, :], kt[:, s + i, :], rhs_q,
                                start=True, stop=True, skip_group_check=True,
                            )
                        pg = p_pool.tile([P, G, QB], BF16, tag="pg")
                        nc.scalar.activation(
                            pg[:], stg[:],
                            mybir.ActivationFunctionType.Exp, scale=0.25,
                        )
                        if jst["prev"] is not None:
                            _emit_pv(nc, jst["ot"], vaug, jst["prev"])
                        jst["prev"] = (pg, [s, s + 1])

                    return run

                def diag_group(a):
                    def run():
                        if jst["ot"] is None:
                            jst["ot"] = ot_pool.tile(
                                [P, 4, P], F32, tag="ot", name="ot"
                            )
                        if a == 0:
                            jst["pgd"] = p_pool.tile(
                                [P, 4, QB], BF16, tag="pgd", name="pgd"
                            )
                            # the masked-out region read by the mask ops below
                            # is never written by the exp; zero it so those
                            # reads are initialised
                            nc.gpsimd.memset(jst["pgd"][:, 2:4, : 2 * P], 0.0)
                        pgd = jst["pgd"]
                        # columns m < 256a of tiles (2a, 2a+1) are fully
                        # masked: skip their QK matmul + exp; affine_select /
                        # dmask below zero-fill that (otherwise stale) region.
                        c0 = 2 * P * a
                        stg = st_pool.tile([P, G, QB], F32, tag="stg")
                        for i in range(G):
                            nc.tensor.matmul(
                                stg[:, i, c0:],
                                kt[:, 4 * mb + 2 * a + i, :],
                                qt[:, 4 * mb + 2 * a : 4 * mb + 4, :],
                                start=True, stop=True, skip_group_check=True,
                            )
                        nc.scalar.activation(
                            pgd[:, 2 * a : 2 * a + 2, c0:], stg[:, :, c0:],
                            mybir.ActivationFunctionType.Exp, scale=0.25,
                        )
                        # keep pgd[n, jj, m] iff m - n - 128 jj >= 0: one DVE
                        # mul by the const mask pair (2-byte dtype, 2x mode)
                        nc.vector.tensor_mul(
                            out=pgd[:, 2 * a : 2 * a + 2, :],
                            in0=pgd[:, 2 * a : 2 * a + 2, :],
                            in1=dmask[a][:],
                        )

                    return run

                def pv_epilogue():
                    ot, pgd = jst["ot"], jst["pgd"]
                    if jst["prev"] is not None:
                        _emit_pv(nc, ot, vaug, jst["prev"])
                    # diagonal PV: query sub-tile j only takes contributions
                    # from diag tiles jj <= j (the rest are fully masked)
                    # PSUM zero-region semantics: start=True resets the whole
                    # 2KB bank, so only the very FIRST matmul into the ot bank
                    # may set it; later sub-tiles' first writes land on
                    # pending-zero bytes and overwrite (not accumulate).
                    for jj in range(4):
                        for j in range(jj, 4):
                            nc.tensor.matmul(
                                ot[:, j, :DE],
                                pgd[:, jj, j * P : (j + 1) * P],
                                vaug[:, 4 * mb + jj, :],
                                start=(nsub == 0 and jj == 0 and j == 0),
                                stop=(jj == j),
                                skip_group_check=True,
                            )
                    # epilogue: normalize + store (output is already in
                    # [query, d] orientation -- no transpose needed)
                    linv = epi_pool.tile([P, 4], F32, tag="linv")
                    nc.vector.reciprocal(linv[:], ot[:, :, D])
                    o_sb = epi_pool.tile([P, 4, D], F32, tag="o_sb")
                    nc.vector.tensor_mul(
                        out=o_sb[:],
                        in0=ot[:, :, :D],
                        in1=linv[:, :, None].to_broadcast((P, 4, D)),
                    )
                    nc.sync.dma_start(
                        out[h, mb * QB : (mb + 1) * QB, :].rearrange(
                            "(j p) d -> p j d", p=P
                        ),
                        o_sb[:],
                    )

                chunks = [sub_group(s) for s in range(0, nsub, G)]
                chunks += [diag_group(0), diag_group(1), pv_epilogue]
                return chunks

            # ---- software-pipelined emission: depth-2 job interleave ----
            # head 0: emit only the first two quarters' setup up front, drip
            # the rest between job chunks so the first QK isn't queued behind
            # every transpose on PE.  Emission order defines dependencies, so
            # job (0,mb) must have its quarters' setup emitted first:
            # h0 chunk list is [allocs, prep0, ktr0, qtr0, dmasks,
            # (prep,ktr,qtr) x 3]; job (0,mb) needs the first 5+3*mb chunks.
            setup_q = {0: list(setup_chunks(0))}
            n0 = len(setup_q[0])
            for _ in range(8):
                setup_q[0].pop(0)()
            # later heads' setup chunks, dripped in ~1.5 heads ahead of use
            setup_q[1] = list(setup_chunks(1))

            def drip_one():
                for hh in sorted(setup_q):
                    if setup_q[hh]:
                        setup_q[hh].pop(0)()
                        return

            jobs = [(h, mb) for h in range(HPC) for mb in range(MBS)]
            active = []           # up to 2 jobs' chunk queues
            ji = 0
            while active or ji < len(jobs):
                while len(active) < 2 and ji < len(jobs):
                    h, mb = jobs[ji]
                    if h == 0:
                        while n0 - len(setup_q[0]) < 5 + 3 * mb:
                            setup_q[0].pop(0)()
                    else:
                        # head h's setup must be fully emitted before its
                        # first job
                        if mb == 0:
                            for c in setup_q.get(h, []):
                                c()
                            setup_q[h] = []
                    if mb == 0 and h + 1 < HPC and h + 1 not in setup_q:
                        setup_q[h + 1] = list(setup_chunks(h + 1))
                    active.append(job_chunks(h, mb))
                    ji += 1
                for q_ in list(active):
                    q_.pop(0)()
                drip_one()
                drip_one()
                active = [q_ for q_ in active if q_]
            for hh in sorted(setup_q):
                for c in setup_q[hh]:
                    c()

    nc.compile()
    return nc


def _emit_pv(nc, ot, vaug, group):
    """PV for a full (unmasked) pair of key tiles: pg tiles are stationary
    [128 key, 128 query] operands, vaug [128 key, 65] moves."""
    pg, tiles = group
    for i, nt in enumerate(tiles):
        for j in range(4):
            # start only on the bank's very first matmul (see pv_epilogue)
            nc.tensor.matmul(
                ot[:, j, : D + 1],
                pg[:, i, j * P : (j + 1) * P],
                vaug[:, nt, :],
                start=(nt == 0 and j == 0),
                stop=False,
                skip_group_check=True,
            )


_NC = None


def _get_nc():
    global _NC
    if _NC is None:
        _NC = build_nc()
    return _NC


def kernel(q: np.ndarray, k: np.ndarray, v: np.ndarray) -> np.ndarray:
    from concourse.bass_utils import run_bass_kernel_spmd

    nc = _get_nc()
    qf = np.ascontiguousarray(np.asarray(q, dtype=np.float32).reshape(B * H, N, D))
    kf = np.ascontiguousarray(np.asarray(k, dtype=np.float32).reshape(B * H, N, D))
    vf = np.ascontiguousarray(np.asarray(v, dtype=np.float32).reshape(B * H, N, D))
    in_maps = [
        {
            "q": np.ascontiguousarray(qf[c * HPC : (c + 1) * HPC]),
            "k": np.ascontiguousarray(kf[c * HPC : (c + 1) * HPC]),
            "v": np.ascontiguousarray(vf[c * HPC : (c + 1) * HPC]),
        }
        for c in range(NCORES)
    ]
    res = run_bass_kernel_spmd(nc, in_maps, core_ids=list(range(NCORES)))
    outs = [res.results[c]["out"] for c in range(NCORES)]
    return np.concatenate(outs, axis=0).reshape(B, H, N, D)


if __name__ == "__main__":
    rng = np.random.default_rng(0)
    qq = rng.standard_normal((B, H, N, D), dtype=np.float32)
    kk = rng.standard_normal((B, H, N, D), dtype=np.float32)
    vv = rng.standard_normal((B, H, N, D), dtype=np.float32)
    o = kernel(q=qq, k=kk, v=vv)
    print("kernel ran, out shape", o.shape, "finite:", np.isfinite(o).all())


# revision 51
# speedup vs baseline: 1.2617x; 1.0289x over previous
"""RBF-kernel causal attention on 8 Trainium2 NeuronCores.

B=2, H=16, N=2048, D=64. Shards the 32 (b,h) attention instances across 8
cores (4 heads per core). Math notes:

  logits = -relu(||q-k||^2)/sqrt(D); relu is a no-op (||q-k||^2 >= 0 up to
  rounding), and softmax is invariant to per-query offsets, so
      softmax_n(-(qsq_m + ksq_n - 2 qk)/8) == softmax_n(qk/4 - ksq_n/8)
  The per-key term is folded into the QK matmul itself: k tiles are extended
  with a 65th row holding ksq_n and q tiles with a 65th row holding -0.5, so
  the 65-partition contraction directly yields qk - ksq/2, and
      P = exp(0.25 * (K Q^T - ksq/2))        in a [key, query] layout.
  V is extended with a ones column (vaug, bf16); the PV step runs P tiles as
  the STATIONARY operand ([128 key, 128 query] bf16) against vaug as the
  moving operand ([128 key, 65]), accumulating O[query, d] | l[query] directly
  in natural orientation -- no output transpose, and only 65 moving rows per
  key tile.  Final output O[m,d] = acc[m,d] / l[m].

Emission is manually software-pipelined: head h+1's setup chunks (transposes,
ksq, vaug build) are interleaved between head h's query blocks so the tile
scheduler (limited lookahead) can overlap them.
"""

import sys

if "/opt/trn_rl_repo" not in sys.path:
    sys.path.insert(0, "/opt/trn_rl_repo")

import numpy as np

import concourse.bacc as bacc
import concourse.mybir as mybir
import concourse.tile as tile
B, H, N, D = 2, 16, 2048, 64
NCORES = 8
HPC = (B * H) // NCORES  # heads per core = 4
P = 128                  # partitions
NT = N // P              # key tiles per head = 16
QB = 512                 # query block = 4 query sub-tiles of 128
MBS = N // QB            # query blocks per head = 4
G = 2                    # key tiles per exp/ACT group (2 PSUM banks)
DE = D + 1               # extended depth (65): ksq row / ones column

F32 = mybir.dt.float32
# float32r = relaxed-precision fp32 matmul (1 cycle/row at moving dim >= 256
# instead of 4 for float32); bit-identical data to f32.
MM_DT = mybir.dt.float32r
BF16 = mybir.dt.bfloat16


def build_nc():
    nc = bacc.Bacc("TRN2", target_bir_lowering=False, debug=False)
    q = nc.dram_tensor("q", [HPC, N, D], F32, kind="ExternalInput")
    k = nc.dram_tensor("k", [HPC, N, D], F32, kind="ExternalInput")
    v = nc.dram_tensor("v", [HPC, N, D], F32, kind="ExternalInput")
    out = nc.dram_tensor("out", [HPC, N, D], F32, kind="ExternalOutput")

    with tile.TileContext(nc) as tc:
        with (
            tc.tile_pool(name="const", bufs=1) as const_pool,
            tc.tile_pool(name="loads", bufs=1) as load_pool,
            tc.tile_pool(name="head", bufs=2) as head_pool,
            tc.tile_pool(name="work", bufs=4) as work_pool,
            tc.tile_pool(name="p", bufs=4) as p_pool,
            tc.tile_pool(name="epi", bufs=6) as epi_pool,
            tc.tile_pool(name="st", bufs=2, space="PSUM") as st_pool,
            tc.tile_pool(name="tpp", bufs=2, space="PSUM") as tp_pool,
            tc.tile_pool(name="otp", bufs=2, space="PSUM") as ot_pool,
        ):
            # identity on DVE (Pool is busy with other startup memsets) and a
            # warm-up transpose right behind it: the PE clock ramps to full
            # speed only after ~3us of busy history, so starting the ramp at
            # ~1us makes the real transposes and first QKs run 2x faster
            identity = const_pool.tile([P, P], F32)
            nc.vector.memset(identity[:], 0.0)
            nc.gpsimd.affine_select(
                out=identity[:], in_=identity[:],
                compare_op=mybir.AluOpType.not_equal, fill=1.0,
                base=0, pattern=[[-1, P]], channel_multiplier=1,
            )
            wtp = tp_pool.tile([DE, 4, P], F32, tag="tp", name="wtp")
            nc.tensor.transpose(wtp[:, 0, :], identity[:, :DE], identity[:])
            # triangular causal mask for the diagonal squares: the PV step
            # only reads pgd[:, jj, j*128:(j+1)*128] for j >= jj, and only the
            # j == jj square intersects the causal boundary -- so one shared
            # [128, 128] mask (keep iff m - n >= 0) covers every diag tile.
            tri = const_pool.tile([P, P], BF16, tag="tri", name="tri")
            nc.gpsimd.memset(tri[:], 1.0)
            nc.gpsimd.affine_select(
                out=tri[:], in_=tri[:],
                compare_op=mybir.AluOpType.is_ge, fill=0.0,
                base=0, pattern=[[1, P]], channel_multiplier=-1,
            )

            # prefetch every head's inputs up front: no-wait DMAs stream in
            # the background while compute proceeds.  k/q land in the low 64
            # columns of 65-wide extended tiles (col 64 is filled on-chip).
            kexts, qexts, vtmps = [], [], []
            for h in range(HPC):
                kext = load_pool.tile([P, NT, DE], F32, tag=f"kext{h}")
                qext = load_pool.tile([P, NT, DE], F32, tag=f"qext{h}")
                vtmp = load_pool.tile([P, NT, D], F32, tag=f"vtmp{h}")
                kq = k[h].rearrange("(t p) d -> p t d", p=P)
                qq = q[h].rearrange("(t p) d -> p t d", p=P)
                vq = v[h].rearrange("(t p) d -> p t d", p=P)
                if h == 0:
                    # quarter-granular, k/q prioritized so the first
                    # transposes can start after ~2 DMAs, v one quarter behind
                    order = [("q", 0, 4), ("k", 0, 2), ("k", 2, 2),
                             ("k", 4, 4), ("q", 4, 4), ("v", 0, 4),
                             ("k", 8, 4), ("q", 8, 4), ("v", 4, 4),
                             ("k", 12, 4), ("q", 12, 4), ("v", 8, 4),
                             ("v", 12, 4)]
                    for which, t0, nt in order:
                        ts = slice(t0, t0 + nt)
                        if which == "k":
                            nc.sync.dma_start(kext[:, ts, :D], kq[:, ts, :])
                        elif which == "q":
                            nc.sync.dma_start(qext[:, ts, :D], qq[:, ts, :])
                        else:
                            nc.sync.dma_start(vtmp[:, ts, :], vq[:, ts, :])
                # heads >= 1 issue their DMAs lazily from prep_chunk: a
                # whole-head DMA would monopolize the DMA engines for ~3us
                # and starve the per-job output stores queued behind it
                kexts.append(kext)
                qexts.append(qext)
                vtmps.append(vtmp)

            heads = [{} for _ in range(HPC)]

            def setup_chunks(h):
                """Emission chunks for head h's setup, in dependency order."""
                st = heads[h]
                kext, qext, vtmp = kexts[h], qexts[h], vtmps[h]

                def allocs():
                    # kt/qt: transposed 65-row operands [d(+ksq | -0.5), keys]
                    st["kt"] = head_pool.tile([DE, NT, P], MM_DT, tag="kt", name="kt")
                    st["qt"] = head_pool.tile([DE, NT, P], MM_DT, tag="qt", name="qt")
                    st["vaug"] = head_pool.tile(
                        [P, NT, DE], BF16, tag="vaug", name="vaug"
                    )
                    # constant 65th rows: q gets -0.5 (so ksq*q65 = -ksq/2),
                    # vaug gets the ones column for the softmax denominator
                    nc.gpsimd.memset(qext[:, :, D], -0.5)
                    nc.gpsimd.memset(st["vaug"][:, :, D], 1.0)

                def prep_chunk(t0, nt=4):
                    # ksq + vaug build for tiles [t0, t0+nt), square on Pool
                    # (DVE stays free for the per-job mask/copy work): runs as
                    # soon as those tiles of k and v have landed.  Head 0
                    # squares on DVE instead -- it gates the whole startup and
                    # Pool is busy building the identity/tri consts then.
                    sq_eng = nc.vector if h == 0 else nc.gpsimd

                    def run():
                        ts = slice(t0, t0 + nt)
                        if h > 0:
                            # lazy quarter loads (see prefetch note above)
                            kq = k[h].rearrange("(t p) d -> p t d", p=P)
                            qq = q[h].rearrange("(t p) d -> p t d", p=P)
                            vq = v[h].rearrange("(t p) d -> p t d", p=P)
                            nc.sync.dma_start(kext[:, ts, :D], kq[:, ts, :])
                            nc.sync.dma_start(qext[:, ts, :D], qq[:, ts, :])
                            nc.sync.dma_start(vtmp[:, ts, :], vq[:, ts, :])
                        ktmp = work_pool.tile([P, 4, D], F32, tag="ktmp")
                        sq_eng.tensor_mul(
                            out=ktmp[:, :nt], in0=kext[:, ts, :D], in1=kext[:, ts, :D]
                        )
                        nc.vector.tensor_reduce(
                            kext[:, ts, D], ktmp[:, :nt],
                            axis=mybir.AxisListType.X, op=mybir.AluOpType.add,
                        )
                        nc.gpsimd.tensor_copy(
                            out=st["vaug"][:, ts, :D], in_=vtmp[:, ts, :]
                        )

                    return run

                def tr_group(which, t0, nt=4):
                    # nt transposes: [128 n, 65] -> [65, 128 n]
                    def run():
                        src = kext if which == "k" else qext
                        dst = st["kt" if which == "k" else "qt"]
                        tp = tp_pool.tile([DE, 4, P], F32, tag="tp", name="tp")
                        for j in range(nt):
                            nc.tensor.transpose(
                                tp[:, j, :], src[:, t0 + j, :], identity[:]
                            )
                        nc.vector.tensor_copy(
                            out=dst[:, t0 : t0 + nt, :], in_=tp[:, :nt]
                        )

                    return run

                def ktr_group(g):
                    return tr_group("k", 4 * g)

                def qtr_group(g):
                    return tr_group("q", 4 * g)

                # query block mb needs kt tiles <= 4mb+3, qt group mb, vaug
                # quarter mb; yield in dependency order.  ksq (prep) gates
                # the k transposes, so preps lead their quarter's groups.
                yield allocs
                if h == 0:
                    # startup critical path: q transposes first (they do not
                    # wait on ksq), k in 2-tile bites so job (0,0)'s QK
                    # (kt tiles 0,1 + qt quarter 0) launches ASAP
                    yield qtr_group(0)
                    yield prep_chunk(0, 2)
                    yield tr_group("k", 0, 2)
                    yield prep_chunk(2, 2)
                    yield tr_group("k", 2, 2)
                    for c in range(1, 4):
                        yield prep_chunk(4 * c)
                        yield qtr_group(c)
                        yield ktr_group(c)
                else:
                    # all preps first: they are cheap, their DMA deps landed
                    # long ago, and everything else queues behind them in
                    # each engine's in-order stream
                    for c in range(4):
                        yield prep_chunk(4 * c)
                    for c in range(4):
                        yield ktr_group(c)
                        yield qtr_group(c)

            def job_chunks(h, mb):
                """Chunks of one (head, query-block) job, for interleaving."""
                kt, qt, vaug = heads[h]["kt"], heads[h]["qt"], heads[h]["vaug"]
                nsub = 4 * mb          # sub-diagonal key tiles
                rhs_q = qt[:, 4 * mb : 4 * mb + 4, :]    # [65, 512]
                jst = {"prev": None, "ot": None}

                def sub_group(s):
                    def run():
                        if jst["ot"] is None:
                            jst["ot"] = ot_pool.tile(
                                [P, 4, P], F32, tag="ot", name="ot"
                            )
                        stg = st_pool.tile([P, G, QB], F32, tag="stg")
                        for i in range(G):
                            nc.tensor.matmul(
                                stg[:, i, :], kt[:, s + i, :], rhs_q,
                                start=True, stop=True, skip_group_check=True,
                            )
                        pg = p_pool.tile([P, G, QB], BF16, tag="pg")
                        nc.scalar.activation(
                            pg[:], stg[:],
                            mybir.ActivationFunctionType.Exp, scale=0.25,
                        )
                        if jst["prev"] is not None:
                            _emit_pv(nc, jst["ot"], vaug, jst["prev"])
                        jst["prev"] = (pg, [s, s + 1])

                    return run

                def diag_group(a):
                    def run():
                        if jst["ot"] is None:
                            jst["ot"] = ot_pool.tile(
                                [P, 4, P], F32, tag="ot", name="ot"
                            )
                        if a == 0:
                            jst["pgd"] = p_pool.tile(
                                [P, 4, QB], BF16, tag="pgd", name="pgd"
                            )
                        pgd = jst["pgd"]
                        # columns m < 256a of tiles (2a, 2a+1) are fully
                        # masked: skip their QK matmul + exp; affine_select /
                        # dmask below zero-fill that (otherwise stale) region.
                        c0 = 2 * P * a
                        stg = st_pool.tile([P, G, QB], F32, tag="stg")
                        for i in range(G):
                            nc.tensor.matmul(
                                stg[:, i, c0:],
                                kt[:, 4 * mb + 2 * a + i, :],
                                qt[:, 4 * mb + 2 * a : 4 * mb + 4, :],
                                start=True, stop=True, skip_group_check=True,
                            )
                        nc.scalar.activation(
                            pgd[:, 2 * a : 2 * a + 2, c0:], stg[:, :, c0:],
                            mybir.ActivationFunctionType.Exp, scale=0.25,
                        )
                        # causal-mask each tile's diagonal 128x128 square (the
                        # only read region crossing the boundary)
                        for i in (0, 1):
                            jj = 2 * a + i
                            nc.vector.tensor_mul(
                                out=pgd[:, jj, jj * P : (jj + 1) * P],
                                in0=pgd[:, jj, jj * P : (jj + 1) * P],
                                in1=tri[:],
                            )

                    return run

                def pv_epilogue():
                    ot, pgd = jst["ot"], jst["pgd"]
                    if jst["prev"] is not None:
                        _emit_pv(nc, ot, vaug, jst["prev"])
                    # diagonal PV: query sub-tile j only takes contributions
                    # from diag tiles jj <= j (the rest are fully masked)
                    # PSUM zero-region semantics: start=True resets the whole
                    # 2KB bank, so only the very FIRST matmul into the ot bank
                    # may set it; later sub-tiles' first writes land on
                    # pending-zero bytes and overwrite (not accumulate).
                    for jj in range(4):
                        for j in range(jj, 4):
                            nc.tensor.matmul(
                                ot[:, j, :DE],
                                pgd[:, jj, j * P : (j + 1) * P],
                                vaug[:, 4 * mb + jj, :],
                                start=(nsub == 0 and jj == 0 and j == 0),
                                stop=(jj == j),
                                skip_group_check=True,
                            )
                    # epilogue: normalize + store (output is already in
                    # [query, d] orientation -- no transpose needed)
                    linv = epi_pool.tile([P, 4], F32, tag="linv")
                    nc.vector.reciprocal(linv[:], ot[:, :, D])
                    o_sb = epi_pool.tile([P, 4, D], F32, tag="o_sb")
                    nc.vector.tensor_mul(
                        out=o_sb[:],
                        in0=ot[:, :, :D],
                        in1=linv[:, :, None].to_broadcast((P, 4, D)),
                    )
                    nc.sync.dma_start(
                        out[h, mb * QB : (mb + 1) * QB, :].rearrange(
                            "(j p) d -> p j d", p=P
                        ),
                        o_sb[:],
                    )

                chunks = [sub_group(s) for s in range(0, nsub, G)]
                chunks += [diag_group(0), diag_group(1), pv_epilogue]
                return chunks

            # ---- software-pipelined emission: depth-2 job interleave ----
            # head 0: emit only the first two quarters' setup up front, drip
            # the rest between job chunks so the first QK isn't queued behind
            # every transpose on PE.  Emission order defines dependencies, so
            # job (0,mb) must have its quarters' setup emitted first:
            # h0 chunk list is [allocs, qtr0, (prep,ktr) x 2 two-tile bites,
            # (prep,ktr,qtr) x 3]; job (0,mb) needs the first 6+3*mb chunks.
            setup_q = {0: list(setup_chunks(0))}
            n0 = len(setup_q[0])
            for _ in range(6):
                setup_q[0].pop(0)()
            # later heads' setup chunks, dripped in ~1.5 heads ahead of use
            setup_q[1] = list(setup_chunks(1))

            def drip_one():
                for hh in sorted(setup_q):
                    if setup_q[hh]:
                        setup_q[hh].pop(0)()
                        return

            jobs = [(h, mb) for h in range(HPC) for mb in range(MBS)]
            active = []           # up to 2 jobs' chunk queues
            ji = 0
            while active or ji < len(jobs):
                while len(active) < 2 and ji < len(jobs):
                    h, mb = jobs[ji]
                    if h == 0:
                        while n0 - len(setup_q[0]) < 6 + 3 * mb:
                            setup_q[0].pop(0)()
                    else:
                        # head h's setup must be fully emitted before its
                        # first job
                        if mb == 0:
                            for c in setup_q.get(h, []):
                                c()
                            setup_q[h] = []
                    if mb == 0 and h + 1 < HPC and h + 1 not in setup_q:
                        setup_q[h + 1] = list(setup_chunks(h + 1))
                    active.append(job_chunks(h, mb))
                    ji += 1
                for q_ in list(active):
                    q_.pop(0)()
                drip_one()
                drip_one()
                active = [q_ for q_ in active if q_]
            for hh in sorted(setup_q):
                for c in setup_q[hh]:
                    c()

    nc.compile()
    return nc


def _emit_pv(nc, ot, vaug, group):
    """PV for a full (unmasked) pair of key tiles: pg tiles are stationary
    [128 key, 128 query] operands, vaug [128 key, 65] moves."""
    pg, tiles = group
    for i, nt in enumerate(tiles):
        for j in range(4):
            # start only on the bank's very first matmul (see pv_epilogue)
            nc.tensor.matmul(
                ot[:, j, : D + 1],
                pg[:, i, j * P : (j + 1) * P],
                vaug[:, nt, :],
                start=(nt == 0 and j == 0),
                stop=False,
                skip_group_check=True,
            )


_NC = None


def _get_nc():
    global _NC
    if _NC is None:
        _NC = build_nc()
    return _NC


def kernel(q: np.ndarray, k: np.ndarray, v: np.ndarray) -> np.ndarray:
    from concourse.bass_utils import run_bass_kernel_spmd

    nc = _get_nc()
    qf = np.ascontiguousarray(np.asarray(q, dtype=np.float32).reshape(B * H, N, D))
    kf = np.ascontiguousarray(np.asarray(k, dtype=np.float32).reshape(B * H, N, D))
    vf = np.ascontiguousarray(np.asarray(v, dtype=np.float32).reshape(B * H, N, D))
    in_maps = [
        {
            "q": np.ascontiguousarray(qf[c * HPC : (c + 1) * HPC]),
            "k": np.ascontiguousarray(kf[c * HPC : (c + 1) * HPC]),
            "v": np.ascontiguousarray(vf[c * HPC : (c + 1) * HPC]),
        }
        for c in range(NCORES)
    ]
    res = run_bass_kernel_spmd(nc, in_maps, core_ids=list(range(NCORES)))
    outs = [res.results[c]["out"] for c in range(NCORES)]
    return np.concatenate(outs, axis=0).reshape(B, H, N, D)


if __name__ == "__main__":
    rng = np.random.default_rng(0)
    qq = rng.standard_normal((B, H, N, D), dtype=np.float32)
    kk = rng.standard_normal((B, H, N, D), dtype=np.float32)
    vv = rng.standard_normal((B, H, N, D), dtype=np.float32)
    o = kernel(q=qq, k=kk, v=vv)
    print("kernel ran, out shape", o.shape, "finite:", np.isfinite(o).all())


# revision 58
# speedup vs baseline: 1.2660x; 1.0035x over previous
"""RBF-kernel causal attention on 8 Trainium2 NeuronCores.

B=2, H=16, N=2048, D=64. Shards the 32 (b,h) attention instances across 8
cores (4 heads per core). Math notes:

  logits = -relu(||q-k||^2)/sqrt(D); relu is a no-op (||q-k||^2 >= 0 up to
  rounding), and softmax is invariant to per-query offsets, so
      softmax_n(-(qsq_m + ksq_n - 2 qk)/8) == softmax_n(qk/4 - ksq_n/8)
  The per-key term is folded into the QK matmul itself: k tiles are extended
  with a 65th row holding ksq_n and q tiles with a 65th row holding -0.5, so
  the 65-partition contraction directly yields qk - ksq/2, and
      P = exp(0.25 * (K Q^T - ksq/2))        in a [key, query] layout.
  V is extended with a ones column (vaug, bf16); the PV step runs P tiles as
  the STATIONARY operand ([128 key, 128 query] bf16) against vaug as the
  moving operand ([128 key, 65]), accumulating O[query, d] | l[query] directly
  in natural orientation -- no output transpose, and only 65 moving rows per
  key tile.  Final output O[m,d] = acc[m,d] / l[m].

Emission is manually software-pipelined: head h+1's setup chunks (transposes,
ksq, vaug build) are interleaved between head h's query blocks so the tile
scheduler (limited lookahead) can overlap them.
"""

import sys

if "/opt/trn_rl_repo" not in sys.path:
    sys.path.insert(0, "/opt/trn_rl_repo")

import numpy as np

import concourse.bacc as bacc
import concourse.mybir as mybir
import concourse.tile as tile
B, H, N, D = 2, 16, 2048, 64
NCORES = 8
HPC = (B * H) // NCORES  # heads per core = 4
P = 128                  # partitions
NT = N // P              # key tiles per head = 16
QB = 512                 # query block = 4 query sub-tiles of 128
MBS = N // QB            # query blocks per head = 4
G = 2                    # key tiles per exp/ACT group (2 PSUM banks)
DE = D + 1               # extended depth (65): ksq row / ones column

F32 = mybir.dt.float32
# float32r = relaxed-precision fp32 matmul (1 cycle/row at moving dim >= 256
# instead of 4 for float32); bit-identical data to f32.
MM_DT = mybir.dt.float32r
BF16 = mybir.dt.bfloat16


def build_nc():
    nc = bacc.Bacc("TRN2", target_bir_lowering=False, debug=False)
    q = nc.dram_tensor("q", [HPC, N, D], F32, kind="ExternalInput")
    k = nc.dram_tensor("k", [HPC, N, D], F32, kind="ExternalInput")
    v = nc.dram_tensor("v", [HPC, N, D], F32, kind="ExternalInput")
    out = nc.dram_tensor("out", [HPC, N, D], F32, kind="ExternalOutput")

    with tile.TileContext(nc) as tc:
        with (
            tc.tile_pool(name="const", bufs=1) as const_pool,
            tc.tile_pool(name="loads", bufs=1) as load_pool,
            tc.tile_pool(name="head", bufs=2) as head_pool,
            tc.tile_pool(name="work", bufs=4) as work_pool,
            tc.tile_pool(name="p", bufs=4) as p_pool,
            tc.tile_pool(name="epi", bufs=6) as epi_pool,
            tc.tile_pool(name="st", bufs=2, space="PSUM") as st_pool,
            tc.tile_pool(name="tpp", bufs=2, space="PSUM") as tp_pool,
            tc.tile_pool(name="otp", bufs=2, space="PSUM") as ot_pool,
        ):
            # identity on DVE (Pool is busy with other startup memsets) and a
            # warm-up transpose right behind it: the PE clock ramps to full
            # speed only after ~3us of busy history, so starting the ramp at
            # ~1us makes the real transposes and first QKs run 2x faster
            identity = const_pool.tile([P, P], F32)
            nc.vector.memset(identity[:], 0.0)
            nc.gpsimd.affine_select(
                out=identity[:], in_=identity[:],
                compare_op=mybir.AluOpType.not_equal, fill=1.0,
                base=0, pattern=[[-1, P]], channel_multiplier=1,
            )
            wtp = tp_pool.tile([DE, 4, P], F32, tag="tp", name="wtp")
            nc.tensor.transpose(wtp[:, 0, :], identity[:, :DE], identity[:])
            # triangular causal mask for the diagonal squares: the PV step
            # only reads pgd[:, jj, j*128:(j+1)*128] for j >= jj, and only the
            # j == jj square intersects the causal boundary -- so one shared
            # [128, 128] mask (keep iff m - n >= 0) covers every diag tile.
            tri = const_pool.tile([P, P], BF16, tag="tri", name="tri")
            nc.gpsimd.memset(tri[:], 1.0)
            nc.gpsimd.affine_select(
                out=tri[:], in_=tri[:],
                compare_op=mybir.AluOpType.is_ge, fill=0.0,
                base=0, pattern=[[1, P]], channel_multiplier=-1,
            )

            # prefetch every head's inputs up front: no-wait DMAs stream in
            # the background while compute proceeds.  k/q land in the low 64
            # columns of 65-wide extended tiles (col 64 is filled on-chip).
            kexts, qexts, vtmps = [], [], []
            for h in range(HPC):
                kext = load_pool.tile([P, NT, DE], F32, tag=f"kext{h}")
                qext = load_pool.tile([P, NT, DE], F32, tag=f"qext{h}")
                vtmp = load_pool.tile([P, NT, D], F32, tag=f"vtmp{h}")
                kq = k[h].rearrange("(t p) d -> p t d", p=P)
                qq = q[h].rearrange("(t p) d -> p t d", p=P)
                vq = v[h].rearrange("(t p) d -> p t d", p=P)
                if h == 0:
                    # quarter-granular, k/q prioritized so the first
                    # transposes can start after ~2 DMAs, v one quarter behind
                    order = [("q", 0, 4), ("k", 0, 2), ("k", 2, 2),
                             ("k", 4, 4), ("q", 4, 4), ("v", 0, 4),
                             ("k", 8, 4), ("q", 8, 4), ("v", 4, 4),
                             ("k", 12, 4), ("q", 12, 4), ("v", 8, 4),
                             ("v", 12, 4)]
                    for which, t0, nt in order:
                        ts = slice(t0, t0 + nt)
                        if which == "k":
                            nc.sync.dma_start(kext[:, ts, :D], kq[:, ts, :])
                        elif which == "q":
                            nc.sync.dma_start(qext[:, ts, :D], qq[:, ts, :])
                        else:
                            nc.sync.dma_start(vtmp[:, ts, :], vq[:, ts, :])
                # heads >= 1 issue their DMAs lazily from prep_chunk: a
                # whole-head DMA would monopolize the DMA engines for ~3us
                # and starve the per-job output stores queued behind it
                kexts.append(kext)
                qexts.append(qext)
                vtmps.append(vtmp)

            heads = [{} for _ in range(HPC)]

            def setup_chunks(h):
                """Emission chunks for head h's setup, in dependency order."""
                st = heads[h]
                kext, qext, vtmp = kexts[h], qexts[h], vtmps[h]

                def allocs():
                    # kt/qt: transposed 65-row operands [d(+ksq | -0.5), keys]
                    st["kt"] = head_pool.tile([DE, NT, P], MM_DT, tag="kt", name="kt")
                    st["qt"] = head_pool.tile([DE, NT, P], MM_DT, tag="qt", name="qt")
                    st["vaug"] = head_pool.tile(
                        [P, NT, DE], BF16, tag="vaug", name="vaug"
                    )
                    # constant 65th rows: q gets -0.5 (so ksq*q65 = -ksq/2),
                    # vaug gets the ones column for the softmax denominator
                    nc.gpsimd.memset(qext[:, :, D], -0.5)
                    nc.gpsimd.memset(st["vaug"][:, :, D], 1.0)

                def prep_chunk(t0, nt=4):
                    # ksq + vaug build for tiles [t0, t0+nt), square on Pool
                    # (DVE stays free for the per-job mask/copy work): runs as
                    # soon as those tiles of k and v have landed.  Head 0
                    # squares on DVE instead -- it gates the whole startup and
                    # Pool is busy building the identity/tri consts then.
                    sq_eng = nc.vector if h == 0 else nc.gpsimd

                    def run():
                        ts = slice(t0, t0 + nt)
                        if h > 0:
                            # lazy quarter loads (see prefetch note above)
                            kq = k[h].rearrange("(t p) d -> p t d", p=P)
                            qq = q[h].rearrange("(t p) d -> p t d", p=P)
                            vq = v[h].rearrange("(t p) d -> p t d", p=P)
                            nc.sync.dma_start(kext[:, ts, :D], kq[:, ts, :])
                            nc.sync.dma_start(qext[:, ts, :D], qq[:, ts, :])
                            nc.sync.dma_start(vtmp[:, ts, :], vq[:, ts, :])
                        ktmp = work_pool.tile([P, 4, D], F32, tag="ktmp")
                        sq_eng.tensor_mul(
                            out=ktmp[:, :nt], in0=kext[:, ts, :D], in1=kext[:, ts, :D]
                        )
                        nc.vector.tensor_reduce(
                            kext[:, ts, D], ktmp[:, :nt],
                            axis=mybir.AxisListType.X, op=mybir.AluOpType.add,
                        )
                        nc.gpsimd.tensor_copy(
                            out=st["vaug"][:, ts, :D], in_=vtmp[:, ts, :]
                        )

                    return run

                def tr_group(which, t0, nt=4):
                    # nt transposes: [128 n, 65] -> [65, 128 n]
                    def run():
                        src = kext if which == "k" else qext
                        dst = st["kt" if which == "k" else "qt"]
                        tp = tp_pool.tile([DE, 4, P], F32, tag="tp", name="tp")
                        for j in range(nt):
                            nc.tensor.transpose(
                                tp[:, j, :], src[:, t0 + j, :], identity[:]
                            )
                        nc.vector.tensor_copy(
                            out=dst[:, t0 : t0 + nt, :], in_=tp[:, :nt]
                        )

                    return run

                def ktr_group(g):
                    return tr_group("k", 4 * g)

                def qtr_group(g):
                    return tr_group("q", 4 * g)

                def tr_pairs(c):
                    # 2-tile transpose chunks: shorter PE bursts interleave
                    # with job QKs without starving the exp pipeline
                    for t0 in (4 * c, 4 * c + 2):
                        yield tr_group("q", t0, 2)
                        yield tr_group("k", t0, 2)

                # query block mb needs kt tiles <= 4mb+3, qt group mb, vaug
                # quarter mb; yield in dependency order.  ksq (prep) gates
                # the k transposes, so preps lead their quarter's groups.
                yield allocs
                if h == 0:
                    # startup critical path: q transposes first (they do not
                    # wait on ksq), k in 2-tile bites so job (0,0)'s QK
                    # (kt tiles 0,1 + qt quarter 0) launches ASAP
                    yield qtr_group(0)
                    yield prep_chunk(0, 2)
                    yield tr_group("k", 0, 2)
                    yield prep_chunk(2, 2)
                    yield tr_group("k", 2, 2)
                    for c in range(1, 4):
                        yield prep_chunk(4 * c)
                        yield from tr_pairs(c)
                else:
                    # all preps first: they are cheap, their DMA deps landed
                    # long ago, and everything else queues behind them in
                    # each engine's in-order stream
                    for c in range(4):
                        yield prep_chunk(4 * c)
                    for c in range(4):
                        yield from tr_pairs(c)

            def job_chunks(h, mb):
                """Chunks of one (head, query-block) job, for interleaving."""
                kt, qt, vaug = heads[h]["kt"], heads[h]["qt"], heads[h]["vaug"]
                nsub = 4 * mb          # sub-diagonal key tiles
                rhs_q = qt[:, 4 * mb : 4 * mb + 4, :]    # [65, 512]
                jst = {"prev": None, "ot": None}

                def sub_group(s):
                    def run():
                        if jst["ot"] is None:
                            jst["ot"] = ot_pool.tile(
                                [P, 4, P], F32, tag="ot", name="ot"
                            )
                        stg = st_pool.tile([P, G, QB], F32, tag="stg")
                        for i in range(G):
                            nc.tensor.matmul(
                                stg[:, i, :], kt[:, s + i, :], rhs_q,
                                start=True, stop=True, skip_group_check=True,
                            )
                        pg = p_pool.tile([P, G, QB], BF16, tag="pg")
                        nc.scalar.activation(
                            pg[:], stg[:],
                            mybir.ActivationFunctionType.Exp, scale=0.25,
                        )
                        if jst["prev"] is not None:
                            _emit_pv(nc, jst["ot"], vaug, jst["prev"])
                        jst["prev"] = (pg, [s, s + 1])

                    return run

                def diag_group(a):
                    def run():
                        if jst["ot"] is None:
                            jst["ot"] = ot_pool.tile(
                                [P, 4, P], F32, tag="ot", name="ot"
                            )
                        if a == 0:
                            jst["pgd"] = p_pool.tile(
                                [P, 4, QB], BF16, tag="pgd", name="pgd"
                            )
                        pgd = jst["pgd"]
                        # columns m < 256a of tiles (2a, 2a+1) are fully
                        # masked: skip their QK matmul + exp; affine_select /
                        # dmask below zero-fill that (otherwise stale) region.
                        c0 = 2 * P * a
                        stg = st_pool.tile([P, G, QB], F32, tag="stg")
                        for i in range(G):
                            nc.tensor.matmul(
                                stg[:, i, c0:],
                                kt[:, 4 * mb + 2 * a + i, :],
                                qt[:, 4 * mb + 2 * a : 4 * mb + 4, :],
                                start=True, stop=True, skip_group_check=True,
                            )
                        nc.scalar.activation(
                            pgd[:, 2 * a : 2 * a + 2, c0:], stg[:, :, c0:],
                            mybir.ActivationFunctionType.Exp, scale=0.25,
                        )
                        # causal-mask each tile's diagonal 128x128 square (the
                        # only read region crossing the boundary)
                        for i in (0, 1):
                            jj = 2 * a + i
                            nc.vector.tensor_mul(
                                out=pgd[:, jj, jj * P : (jj + 1) * P],
                                in0=pgd[:, jj, jj * P : (jj + 1) * P],
                                in1=tri[:],
                            )

                    return run

                def pv_epilogue():
                    ot, pgd = jst["ot"], jst["pgd"]
                    if jst["prev"] is not None:
                        _emit_pv(nc, ot, vaug, jst["prev"])
                    # diagonal PV: query sub-tile j only takes contributions
                    # from diag tiles jj <= j (the rest are fully masked)
                    # PSUM zero-region semantics: start=True resets the whole
                    # 2KB bank, so only the very FIRST matmul into the ot bank
                    # may set it; later sub-tiles' first writes land on
                    # pending-zero bytes and overwrite (not accumulate).
                    # The very last job stores in two halves so its final DMA
                    # (a ~3.5us latency chain) starts earlier and is smaller.
                    last = (h, mb) == (HPC - 1, MBS - 1)
                    for jj in range(4):
                        for j in range(jj, 4):
                            nc.tensor.matmul(
                                ot[:, j, :DE],
                                pgd[:, jj, j * P : (j + 1) * P],
                                vaug[:, 4 * mb + jj, :],
                                start=(nsub == 0 and jj == 0 and j == 0),
                                stop=(jj == j),
                                skip_group_check=True,
                            )
                        if last and jj == 1:
                            # j sub-tiles 0,1 are complete: flush them now
                            _emit_store(nc, out, epi_pool, ot, h, mb, 0, 2)
                    # epilogue: normalize + store (output is already in
                    # [query, d] orientation -- no transpose needed)
                    if last:
                        _emit_store(nc, out, epi_pool, ot, h, mb, 2, 4)
                    else:
                        _emit_store(nc, out, epi_pool, ot, h, mb, 0, 4)

                chunks = [sub_group(s) for s in range(0, nsub, G)]
                chunks += [diag_group(0), diag_group(1), pv_epilogue]
                return chunks

            # ---- software-pipelined emission: depth-2 job interleave ----
            # head 0: emit only the first two quarters' setup up front, drip
            # the rest between job chunks so the first QK isn't queued behind
            # every transpose on PE.  Emission order defines dependencies, so
            # job (0,mb) must have its quarters' setup emitted first:
            # h0 chunk list is [allocs, qtr0, (prep,ktr) x 2 two-tile bites,
            # (prep, 4 tr pairs) x 3]; job (0,mb) needs the first 6+5*mb.
            setup_q = {0: list(setup_chunks(0))}
            n0 = len(setup_q[0])
            for _ in range(6):
                setup_q[0].pop(0)()
            # later heads' setup chunks, dripped in ~1.5 heads ahead of use
            setup_q[1] = list(setup_chunks(1))

            def drip_one():
                for hh in sorted(setup_q):
                    if setup_q[hh]:
                        setup_q[hh].pop(0)()
                        return

            jobs = [(h, mb) for h in range(HPC) for mb in range(MBS)]
            active = []           # up to 2 jobs' chunk queues
            ji = 0
            while active or ji < len(jobs):
                while len(active) < 2 and ji < len(jobs):
                    h, mb = jobs[ji]
                    if h == 0:
                        while n0 - len(setup_q[0]) < 6 + 5 * mb:
                            setup_q[0].pop(0)()
                    else:
                        # head h's setup must be fully emitted before its
                        # first job
                        if mb == 0:
                            for c in setup_q.get(h, []):
                                c()
                            setup_q[h] = []
                    if mb == 0 and h + 1 < HPC and h + 1 not in setup_q:
                        setup_q[h + 1] = list(setup_chunks(h + 1))
                    active.append(job_chunks(h, mb))
                    ji += 1
                for q_ in list(active):
                    q_.pop(0)()
                drip_one()
                drip_one()
                active = [q_ for q_ in active if q_]
            for hh in sorted(setup_q):
                for c in setup_q[hh]:
                    c()

    nc.compile()
    return nc


def _emit_store(nc, out, epi_pool, ot, h, mb, j0, j1):
    """Normalize query sub-tiles [j0, j1) of the accumulator and DMA them."""
    nj = j1 - j0
    linv = epi_pool.tile([P, 4], F32, tag="linv")
    nc.vector.reciprocal(linv[:, :nj], ot[:, j0:j1, D])
    o_sb = epi_pool.tile([P, 4, D], F32, tag="o_sb")
    nc.vector.tensor_mul(
        out=o_sb[:, :nj],
        in0=ot[:, j0:j1, :D],
        in1=linv[:, :nj, None].to_broadcast((P, nj, D)),
    )
    nc.sync.dma_start(
        out[h, mb * QB + j0 * P : mb * QB + j1 * P, :].rearrange(
            "(j p) d -> p j d", p=P
        ),
        o_sb[:, :nj],
    )


def _emit_pv(nc, ot, vaug, group):
    """PV for a full (unmasked) pair of key tiles: pg tiles are stationary
    [128 key, 128 query] operands, vaug [128 key, 65] moves."""
    pg, tiles = group
    for i, nt in enumerate(tiles):
        for j in range(4):
            # start only on the bank's very first matmul (see pv_epilogue)
            nc.tensor.matmul(
                ot[:, j, : D + 1],
                pg[:, i, j * P : (j + 1) * P],
                vaug[:, nt, :],
                start=(nt == 0 and j == 0),
                stop=False,
                skip_group_check=True,
            )


_NC = None


def _get_nc():
    global _NC
    if _NC is None:
        _NC = build_nc()
    return _NC


def kernel(q: np.ndarray, k: np.ndarray, v: np.ndarray) -> np.ndarray:
    from concourse.bass_utils import run_bass_kernel_spmd

    nc = _get_nc()
    qf = np.ascontiguousarray(np.asarray(q, dtype=np.float32).reshape(B * H, N, D))
    kf = np.ascontiguousarray(np.asarray(k, dtype=np.float32).reshape(B * H, N, D))
    vf = np.ascontiguousarray(np.asarray(v, dtype=np.float32).reshape(B * H, N, D))
    in_maps = [
        {
            "q": np.ascontiguousarray(qf[c * HPC : (c + 1) * HPC]),
            "k": np.ascontiguousarray(kf[c * HPC : (c + 1) * HPC]),
            "v": np.ascontiguousarray(vf[c * HPC : (c + 1) * HPC]),
        }
        for c in range(NCORES)
    ]
    res = run_bass_kernel_spmd(nc, in_maps, core_ids=list(range(NCORES)))
    outs = [res.results[c]["out"] for c in range(NCORES)]
    return np.concatenate(outs, axis=0).reshape(B, H, N, D)


if __name__ == "__main__":
    rng = np.random.default_rng(0)
    qq = rng.standard_normal((B, H, N, D), dtype=np.float32)
    kk = rng.standard_normal((B, H, N, D), dtype=np.float32)
    vv = rng.standard_normal((B, H, N, D), dtype=np.float32)
    o = kernel(q=qq, k=kk, v=vv)
    print("kernel ran, out shape", o.shape, "finite:", np.isfinite(o).all())
